# revision 5
# baseline (speedup 1.0000x reference)
"""Trainium2 Bass kernel for nn_Attention (dense transformer attention block).

Reference computation (per batch b):
  q = BN(wq @ x)  -> (8 heads, 16, 3136)
  k = BN(wk @ x)  -> (8, 16, 3136)
  v = BN(wv @ x)  -> (8, 64, 3136)
  attn = softmax(q^T k)  (scores over 3136x3136 tokens, no scaling)
  o = attn @ v^T -> (8, 64, 3136) -> (512, 56, 56)
  out = BN(wp @ o) -> (256, 56, 56)

Sharding: 8 cores = 2 batches x 4 token-chunks of 784 query tokens.
Each core computes k/v for the full 3136 tokens (cheap) and attention +
output projection for its own 784 query tokens. Zero collectives.

Device algorithm per core (flash-style, f32 end to end):
  - All BN scale factors folded into weights host-side; biases folded via an
    appended ones-row on x (K=257 contraction).
  - S_T[m, n-chunk] = k_blk^T q  (K=16), exp on ACT from PSUM,
    o'[65, n] += v'^T_blk @ exp(S_T_blk)  where v' has an appended ones
    column so row 64 of o' accumulates the softmax denominator.
  - o = o'[0:64] * reciprocal(o'[64]) broadcast via DMA.
  - out = wp_eff @ [o; 1].
"""

import os
import sys

for _p in ("/opt/trn_rl_repo", "/root/.axon_site/_ro/trn_rl_repo"):
    if os.path.isdir(_p) and _p not in sys.path:
        sys.path.insert(0, _p)

import numpy as np

NUM_HEADS = 8
KEY_DIM = 16
D_HEAD = 64
B = 2
C = 256
HH = 56
WW = 56
N = HH * WW          # 3136 tokens
NCHUNK = N // 4      # 784 query tokens per core
NSUB = NCHUNK // 2   # 392, fits one PSUM bank
NB = (N + 127) // 128            # 25 key-blocks
MB_SIZES = [128] * 24 + [64]
KS = [128, 128, 1]               # contraction chunks for K=257
GROUPS = [list(range(g * 3, min(g * 3 + 3, NB))) for g in range(9)]

_GRAPH = None


def _build_graph():
    import concourse.bass as bass  # noqa: F401
    import concourse.mybir as mybir
    import concourse.tile as tile
    from concourse import bacc
    from contextlib import ExitStack

    f32 = mybir.dt.float32
    Exp = mybir.ActivationFunctionType.Exp

    nc = bacc.Bacc("TRN2", target_bir_lowering=False, debug=False, num_devices=8)
    xa_d = nc.dram_tensor("xa", [257, N], f32, kind="ExternalInput").ap()
    xq_d = nc.dram_tensor("xq", [257, NCHUNK], f32, kind="ExternalInput").ap()
    wq_d = nc.dram_tensor("wq", [257, 128], f32, kind="ExternalInput").ap()
    wk_d = nc.dram_tensor("wk", [257, 128], f32, kind="ExternalInput").ap()
    wv_d = nc.dram_tensor("wv", [257, 520], f32, kind="ExternalInput").ap()
    wp_d = nc.dram_tensor("wp", [64, 9, 256], f32, kind="ExternalInput").ap()
    out_d = nc.dram_tensor("out", [256, NCHUNK], f32, kind="ExternalOutput").ap()
    rsd_d = nc.dram_tensor("rsd", [16, NSUB], f32).ap()  # rowsum bounce

    with tile.TileContext(nc) as tc, ExitStack() as stk:
        const = stk.enter_context(tc.tile_pool(name="const", bufs=1))
        xq_sb = const.tile([128, 3, NCHUNK], f32, tag="xq")
        wq_sb = const.tile([128, 3, 128], f32, tag="wq")
        wk_sb = const.tile([128, 3, 128], f32, tag="wk")
        wv_sb = const.tile([128, 3, 520], f32, tag="wv")
        wp_sb = const.tile([64, 9, 256], f32, tag="wp")
        ones_sb = const.tile([1, NCHUNK], f32, tag="ones")
        # per-head 32-aligned base partitions: head h -> (k_lo if h<4 else
        # k_hi) partitions [32*(h%4), 32*(h%4)+16)
        k_lo = const.tile([128, N], f32, tag="klo")
        k_hi = const.tile([128, N], f32, tag="khi")
        q_lo = const.tile([128, NCHUNK], f32, tag="qlo")
        q_hi = const.tile([128, NCHUNK], f32, tag="qhi")
        # v'^T: [m-in-block, block, head-half, 65*hh + (64 v cols + ones col)]
        vT_sb = const.tile([128, NB, 2, 260], f32, tag="vt")
        of_sb = const.tile([64, 8, NCHUNK], f32, tag="of")
        y_sb = const.tile([128, 2, NCHUNK], f32, tag="y")

        for kc in range(3):
            ks, off = KS[kc], 128 * kc
            nc.sync.dma_start(out=wq_sb[0:ks, kc, :], in_=wq_d[off:off + ks, :])
            nc.sync.dma_start(out=wk_sb[0:ks, kc, :], in_=wk_d[off:off + ks, :])
            nc.sync.dma_start(out=wv_sb[0:ks, kc, :], in_=wv_d[off:off + ks, :])
            nc.sync.dma_start(out=xq_sb[0:ks, kc, :], in_=xq_d[off:off + ks, :])
        nc.sync.dma_start(out=wp_sb[:], in_=wp_d[:])
        nc.vector.memset(ones_sb[:], 1.0)

        with tc.tile_pool(name="stageA", bufs=2) as sA, \
             tc.tile_pool(name="psA", bufs=2, space="PSUM") as psA, \
             tc.tile_pool(name="tmpA", bufs=1) as tA:
            k_sb = tA.tile([128, N], f32, tag="ksb")
            q_sb = tA.tile([128, NCHUNK], f32, tag="qsb")
            # q projection (bias folded via ones row of xq)
            for c2 in range(2):
                q_ps = psA.tile([128, 512], f32, tag="qkps")
                for kc in range(3):
                    nc.tensor.matmul(
                        q_ps[0:128, 0:NSUB],
                        wq_sb[0:KS[kc], kc, :],
                        xq_sb[0:KS[kc], kc, c2 * NSUB:(c2 + 1) * NSUB],
                        start=(kc == 0), stop=(kc == 2))
                nc.vector.tensor_copy(
                    q_sb[:, c2 * NSUB:(c2 + 1) * NSUB], q_ps[0:128, 0:NSUB])
            # k projection + v'^T, in 512-column passes over xa
            for p in range(7):
                c0 = 512 * p
                cw = min(512, N - c0)
                xa_t = sA.tile([128, 3, 512], f32, tag="xat")
                for kc in range(3):
                    nc.sync.dma_start(
                        out=xa_t[0:KS[kc], kc, 0:cw],
                        in_=xa_d[128 * kc:128 * kc + KS[kc], c0:c0 + cw])
                k_ps = psA.tile([128, 512], f32, tag="qkps")
                for kc in range(3):
                    nc.tensor.matmul(
                        k_ps[0:128, 0:cw],
                        wk_sb[0:KS[kc], kc, :],
                        xa_t[0:KS[kc], kc, 0:cw],
                        start=(kc == 0), stop=(kc == 2))
                nc.vector.tensor_copy(k_sb[:, c0:c0 + cw], k_ps[0:128, 0:cw])
                for mbi in range(4):
                    mb = 4 * p + mbi
                    if mb >= NB:
                        break
                    pb = MB_SIZES[mb]
                    vt_ps = psA.tile([128, 2, 512], f32, tag="vtps")
                    for half in range(2):
                        for kc in range(3):
                            nc.tensor.matmul(
                                vt_ps[0:pb, half, 0:260],
                                xa_t[0:KS[kc], kc, mbi * 128:mbi * 128 + pb],
                                wv_sb[0:KS[kc], kc, half * 260:(half + 1) * 260],
                                start=(kc == 0), stop=(kc == 2))
                    nc.vector.tensor_copy(
                        vT_sb[0:pb, mb, :, :], vt_ps[0:pb, :, 0:260])
            # regroup heads onto 32-aligned bases
            for h in range(8):
                kt = k_lo if h < 4 else k_hi
                qt = q_lo if h < 4 else q_hi
                bp_ = 32 * (h % 4)
                nc.sync.dma_start(out=kt[bp_:bp_ + 16, :], in_=k_sb[16 * h:16 * h + 16, :])
                nc.sync.dma_start(out=qt[bp_:bp_ + 16, :], in_=q_sb[16 * h:16 * h + 16, :])

        # main attention loop
        with tc.tile_pool(name="pP", bufs=3) as pP, \
             tc.tile_pool(name="pEp", bufs=3) as pEp, \
             tc.tile_pool(name="psS", bufs=2, space="PSUM") as psS, \
             tc.tile_pool(name="psO", bufs=2, space="PSUM") as psO:
            for h in range(8):
                kt = k_lo if h < 4 else k_hi
                qt = q_lo if h < 4 else q_hi
                bp_ = 32 * (h % 4)
                half, hh = h // 4, h % 4
                for c2 in range(2):
                    nc0 = c2 * NSUB
                    o_ps = psO.tile([128, 512], f32, tag="ops")
                    for blocks in GROUPS:
                        s_ps = psS.tile([128, 3, 512], f32, tag="sps")
                        gsz = len(blocks)
                        pb = MB_SIZES[blocks[-1]]
                        for i, mb in enumerate(blocks):
                            pbi = MB_SIZES[mb]
                            nc.tensor.matmul(
                                s_ps[0:pbi, i, 0:NSUB],
                                kt[bp_:bp_ + 16, mb * 128:mb * 128 + pbi],
                                qt[bp_:bp_ + 16, nc0:nc0 + NSUB],
                                start=True, stop=True,
                                tile_position=(bp_, 0))
                        p_sb = pP.tile([128, 3, NSUB], f32, tag="psb")
                        nc.scalar.activation(
                            out=p_sb[0:pb, 0:gsz, :],
                            in_=s_ps[0:pb, 0:gsz, 0:NSUB], func=Exp)
                        for i, mb in enumerate(blocks):
                            pbi = MB_SIZES[mb]
                            nc.tensor.matmul(
                                o_ps[0:65, 0:NSUB],
                                vT_sb[0:pbi, mb, half, 65 * hh:65 * hh + 65],
                                p_sb[0:pbi, i, :],
                                start=(mb == 0), stop=(mb == NB - 1))
                    # softmax denominator: row 64 of o_ps
                    idx = h * 2 + c2
                    rsh = pEp.tile([128, NSUB], f32, tag="rsh")
                    nc.vector.tensor_copy(rsh[64:65, :], o_ps[64:65, 0:NSUB])
                    nc.sync.dma_start(
                        out=rsd_d[idx:idx + 1, :], in_=rsh[64:65, :])
                    rb = pEp.tile([64, NSUB], f32, tag="rb")
                    nc.sync.dma_start(
                        out=rb[0:64, :],
                        in_=rsd_d[idx:idx + 1, :].partition_broadcast(64))
                    rbr = pEp.tile([64, NSUB], f32, tag="rbr")
                    scr = pEp.tile([64, NSUB], f32, tag="scr")
                    nc.vector.reciprocal_approx_accurate(
                        out=rbr[:], in_=rb[0:64, :], scratch=scr[:])
                    nc.vector.tensor_mul(
                        out=of_sb[0:64, h, nc0:nc0 + NSUB],
                        in0=o_ps[0:64, 0:NSUB], in1=rbr[:])

        # output projection: out = wp_eff @ [o; 1]
        with tc.tile_pool(name="psY", bufs=2, space="PSUM") as psY:
            for mo in range(2):
                for c2 in range(2):
                    nc0 = c2 * NSUB
                    y_ps = psY.tile([128, 512], f32, tag="yps")
                    for kc in range(9):
                        if kc < 8:
                            lhsT = wp_sb[0:64, kc, mo * 128:(mo + 1) * 128]
                            rhs = of_sb[0:64, kc, nc0:nc0 + NSUB]
                        else:
                            lhsT = wp_sb[0:1, 8, mo * 128:(mo + 1) * 128]
                            rhs = ones_sb[0:1, nc0:nc0 + NSUB]
                        nc.tensor.matmul(
                            y_ps[0:128, 0:NSUB], lhsT, rhs,
                            start=(kc == 0), stop=(kc == 8))
                    nc.vector.tensor_copy(
                        y_sb[:, mo, nc0:nc0 + NSUB], y_ps[0:128, 0:NSUB])
            for mo in range(2):
                nc.sync.dma_start(
                    out=out_d[mo * 128:(mo + 1) * 128, :], in_=y_sb[:, mo, :])

    nc.compile()
    return nc


def get_graph():
    global _GRAPH
    if _GRAPH is None:
        _GRAPH = _build_graph()
    return _GRAPH


def make_in_maps(x, wq, sq, bq, wk, sk, bk, wv, sv, bv, wp, sp, bp):
    f = np.float32
    x2 = np.asarray(x, f).reshape(B, C, N)
    ones_row = np.ones((1, N), f)
    wq = np.asarray(wq, f); sq = np.asarray(sq, f); bq = np.asarray(bq, f)
    wk = np.asarray(wk, f); sk = np.asarray(sk, f); bk = np.asarray(bk, f)
    wv = np.asarray(wv, f); sv = np.asarray(sv, f); bv = np.asarray(bv, f)
    wp = np.asarray(wp, f); sp = np.asarray(sp, f); bp = np.asarray(bp, f)

    wq_eff = np.concatenate([(wq * sq[:, None]).T, bq[None, :]], 0).astype(f)
    wk_eff = np.concatenate([(wk * sk[:, None]).T, bk[None, :]], 0).astype(f)
    wv_base = wv * sv[:, None]  # (512, 256)
    wv_arr = np.zeros((257, 520), f)
    for h in range(NUM_HEADS):
        col = 260 * (h // 4) + 65 * (h % 4)
        wv_arr[0:256, col:col + 64] = wv_base[64 * h:64 * h + 64, :].T
        wv_arr[256, col:col + 64] = bv[64 * h:64 * h + 64]
        wv_arr[256, col + 64] = 1.0
    wp_eff = (wp * sp[:, None]).T.astype(f)  # (512, 256), row c = 64h+d
    wp_arr = np.zeros((64, 9, 256), f)
    wp_arr[:, 0:8, :] = wp_eff.reshape(8, 64, 256).transpose(1, 0, 2)
    wp_arr[0, 8, :] = bp

    in_maps = []
    for core in range(8):
        b, j = core // 4, core % 4
        xa_full = np.ascontiguousarray(np.concatenate([x2[b], ones_row], 0))
        xq_c = np.ascontiguousarray(xa_full[:, j * NCHUNK:(j + 1) * NCHUNK])
        in_maps.append(dict(xa=xa_full, xq=xq_c, wq=wq_eff, wk=wk_eff,
                            wv=wv_arr, wp=wp_arr))
    return in_maps


def assemble_output(results):
    y = np.zeros((B, C, N), np.float32)
    for core in range(8):
        b, j = core // 4, core % 4
        y[b, :, j * NCHUNK:(j + 1) * NCHUNK] = results[core]["out"]
    return y.reshape(B, C, HH, WW)


def kernel(**inputs):
    from concourse.bass_utils import run_bass_kernel_spmd
    nc = get_graph()
    in_maps = make_in_maps(**inputs)
    res = run_bass_kernel_spmd(nc, in_maps, core_ids=list(range(8)))
    return assemble_output(res.results)


if __name__ == "__main__":
    rng = np.random.default_rng(0)
    ins = dict(
        x=rng.standard_normal((2, 256, 56, 56), np.float32),
        wq=rng.standard_normal((128, 256), np.float32) * 0.05,
        sq=rng.random(128, np.float32),
        bq=rng.standard_normal(128, np.float32) * 0.05,
        wk=rng.standard_normal((128, 256), np.float32) * 0.05,
        sk=rng.random(128, np.float32),
        bk=rng.standard_normal(128, np.float32) * 0.05,
        wv=rng.standard_normal((512, 256), np.float32) * 0.05,
        sv=rng.random(512, np.float32),
        bv=rng.standard_normal(512, np.float32) * 0.05,
        wp=rng.standard_normal((256, 512), np.float32) * 0.05,
        sp=rng.random(256, np.float32),
        bp=rng.standard_normal(256, np.float32) * 0.05,
    )
    out = kernel(**ins)
    print("out", out.shape, out.dtype, float(np.abs(out).mean()))


# revision 7
# speedup vs baseline: 1.0973x; 1.0973x over previous
"""Trainium2 Bass kernel for nn_Attention (dense transformer attention block).

Reference computation (per batch b):
  q = BN(wq @ x)  -> (8 heads, 16, 3136)
  k = BN(wk @ x)  -> (8, 16, 3136)
  v = BN(wv @ x)  -> (8, 64, 3136)
  attn = softmax(q^T k)  (scores over 3136x3136 tokens, no scaling)
  o = attn @ v^T -> (8, 64, 3136) -> (512, 56, 56)
  out = BN(wp @ o) -> (256, 56, 56)

Sharding: 8 cores = 2 batches x 4 token-chunks of 784 query tokens.
Each core computes k/v for the full 3136 tokens (cheap) and attention +
output projection for its own 784 query tokens. Zero collectives.

Device algorithm per core (flash-style, f32 end to end):
  - All BN scale factors folded into weights host-side; biases folded via an
    appended ones-row on x (K=257 contraction).
  - S_T[m, n-chunk] = k_blk^T q  (K=16), exp on ACT from PSUM,
    o'[65, n] += v'^T_blk @ exp(S_T_blk)  where v' has an appended ones
    column so row 64 of o' accumulates the softmax denominator.
  - o = o'[0:64] * reciprocal(o'[64]) broadcast via DMA.
  - out = wp_eff @ [o; 1].
"""

import os
import sys

for _p in ("/opt/trn_rl_repo", "/root/.axon_site/_ro/trn_rl_repo"):
    if os.path.isdir(_p) and _p not in sys.path:
        sys.path.insert(0, _p)

import numpy as np

NUM_HEADS = 8
KEY_DIM = 16
D_HEAD = 64
B = 2
C = 256
HH = 56
WW = 56
N = HH * WW          # 3136 tokens
NCHUNK = N // 4      # 784 query tokens per core
NSUB = NCHUNK // 2   # 392, fits one PSUM bank
NB = (N + 127) // 128            # 25 key-blocks
MB_SIZES = [128] * 24 + [64]
KS = [128, 128, 1]               # contraction chunks for K=257
GROUPS = [list(range(g * 3, min(g * 3 + 3, NB))) for g in range(9)]

_GRAPH = None


def _build_graph():
    import concourse.bass as bass  # noqa: F401
    import concourse.mybir as mybir
    import concourse.tile as tile
    from concourse import bacc
    from contextlib import ExitStack

    f32 = mybir.dt.float32
    Exp = mybir.ActivationFunctionType.Exp

    nc = bacc.Bacc("TRN2", target_bir_lowering=False, debug=False, num_devices=8)
    xa_d = nc.dram_tensor("xa", [257, N], f32, kind="ExternalInput").ap()
    xq_d = nc.dram_tensor("xq", [257, NCHUNK], f32, kind="ExternalInput").ap()
    wq_d = nc.dram_tensor("wq", [257, 128], f32, kind="ExternalInput").ap()
    wk_d = nc.dram_tensor("wk", [257, 128], f32, kind="ExternalInput").ap()
    wv_d = nc.dram_tensor("wv", [257, 520], f32, kind="ExternalInput").ap()
    wp_d = nc.dram_tensor("wp", [64, 9, 256], f32, kind="ExternalInput").ap()
    out_d = nc.dram_tensor("out", [256, NCHUNK], f32, kind="ExternalOutput").ap()
    rsd_d = nc.dram_tensor("rsd", [16, NSUB], f32).ap()  # rowsum bounce

    with tile.TileContext(nc) as tc, ExitStack() as stk:
        const = stk.enter_context(tc.tile_pool(name="const", bufs=1))
        xq_sb = const.tile([128, 3, NCHUNK], f32, tag="xq")
        wq_sb = const.tile([128, 3, 128], f32, tag="wq")
        wk_sb = const.tile([128, 3, 128], f32, tag="wk")
        wv_sb = const.tile([128, 3, 520], f32, tag="wv")
        wp_sb = const.tile([64, 9, 256], f32, tag="wp")
        ones_sb = const.tile([1, NCHUNK], f32, tag="ones")
        # per-head 32-aligned base partitions: head h -> (k_lo if h<4 else
        # k_hi) partitions [32*(h%4), 32*(h%4)+16)
        k_lo = const.tile([128, N], f32, tag="klo")
        k_hi = const.tile([128, N], f32, tag="khi")
        q_lo = const.tile([128, NCHUNK], f32, tag="qlo")
        q_hi = const.tile([128, NCHUNK], f32, tag="qhi")
        # v'^T: [m-in-block, block, head-half, 65*hh + (64 v cols + ones col)]
        vT_sb = const.tile([128, NB, 2, 260], f32, tag="vt")
        of_sb = const.tile([64, 8, NCHUNK], f32, tag="of")
        y_sb = const.tile([128, 2, NCHUNK], f32, tag="y")

        for kc in range(3):
            ks, off = KS[kc], 128 * kc
            nc.sync.dma_start(out=wq_sb[0:ks, kc, :], in_=wq_d[off:off + ks, :])
            nc.sync.dma_start(out=wk_sb[0:ks, kc, :], in_=wk_d[off:off + ks, :])
            nc.sync.dma_start(out=wv_sb[0:ks, kc, :], in_=wv_d[off:off + ks, :])
            nc.sync.dma_start(out=xq_sb[0:ks, kc, :], in_=xq_d[off:off + ks, :])
        nc.sync.dma_start(out=wp_sb[:], in_=wp_d[:])
        nc.vector.memset(ones_sb[:], 1.0)

        with tc.tile_pool(name="stageA", bufs=2) as sA, \
             tc.tile_pool(name="psA", bufs=2, space="PSUM") as psA, \
             tc.tile_pool(name="tmpA", bufs=1) as tA:
            k_sb = tA.tile([128, N], f32, tag="ksb")
            q_sb = tA.tile([128, NCHUNK], f32, tag="qsb")
            # q projection (bias folded via ones row of xq)
            for c2 in range(2):
                q_ps = psA.tile([128, 512], f32, tag="qkps")
                for kc in range(3):
                    nc.tensor.matmul(
                        q_ps[0:128, 0:NSUB],
                        wq_sb[0:KS[kc], kc, :],
                        xq_sb[0:KS[kc], kc, c2 * NSUB:(c2 + 1) * NSUB],
                        start=(kc == 0), stop=(kc == 2))
                nc.vector.tensor_copy(
                    q_sb[:, c2 * NSUB:(c2 + 1) * NSUB], q_ps[0:128, 0:NSUB])
            # k projection + v'^T, in 512-column passes over xa
            for p in range(7):
                c0 = 512 * p
                cw = min(512, N - c0)
                xa_t = sA.tile([128, 3, 512], f32, tag="xat")
                for kc in range(3):
                    nc.sync.dma_start(
                        out=xa_t[0:KS[kc], kc, 0:cw],
                        in_=xa_d[128 * kc:128 * kc + KS[kc], c0:c0 + cw])
                k_ps = psA.tile([128, 512], f32, tag="qkps")
                for kc in range(3):
                    nc.tensor.matmul(
                        k_ps[0:128, 0:cw],
                        wk_sb[0:KS[kc], kc, :],
                        xa_t[0:KS[kc], kc, 0:cw],
                        start=(kc == 0), stop=(kc == 2))
                nc.vector.tensor_copy(k_sb[:, c0:c0 + cw], k_ps[0:128, 0:cw])
                for mbi in range(4):
                    mb = 4 * p + mbi
                    if mb >= NB:
                        break
                    pb = MB_SIZES[mb]
                    vt_ps = psA.tile([128, 2, 512], f32, tag="vtps")
                    for half in range(2):
                        for kc in range(3):
                            nc.tensor.matmul(
                                vt_ps[0:pb, half, 0:260],
                                xa_t[0:KS[kc], kc, mbi * 128:mbi * 128 + pb],
                                wv_sb[0:KS[kc], kc, half * 260:(half + 1) * 260],
                                start=(kc == 0), stop=(kc == 2))
                    nc.vector.tensor_copy(
                        vT_sb[0:pb, mb, :, :], vt_ps[0:pb, :, 0:260])
            # regroup heads onto 32-aligned bases
            for h in range(8):
                kt = k_lo if h < 4 else k_hi
                qt = q_lo if h < 4 else q_hi
                bp_ = 32 * (h % 4)
                nc.sync.dma_start(out=kt[bp_:bp_ + 16, :], in_=k_sb[16 * h:16 * h + 16, :])
                nc.sync.dma_start(out=qt[bp_:bp_ + 16, :], in_=q_sb[16 * h:16 * h + 16, :])

        # main attention loop — head PAIRS with different PE row groups
        # interleaved so the PE never stalls on one head's exp and LDWEIGHTS
        # can pull ahead (different row_grp).
        PAIRS = [(0, 2), (1, 3), (4, 6), (5, 7)]
        with tc.tile_pool(name="pP", bufs=4) as pP, \
             tc.tile_pool(name="pEp", bufs=3) as pEp, \
             tc.tile_pool(name="psS", bufs=2, space="PSUM") as psS, \
             tc.tile_pool(name="psO", bufs=2, space="PSUM") as psO:
            for pair in PAIRS:
                kts = [k_lo if h < 4 else k_hi for h in pair]
                qts = [q_lo if h < 4 else q_hi for h in pair]
                bps = [32 * (h % 4) for h in pair]
                for c2 in range(2):
                    nc0 = c2 * NSUB
                    o_ps2 = [psO.tile([128, 512], f32, tag="ops", name=f"ops{e}") for e in range(2)]
                    for blocks in GROUPS:
                        gsz = len(blocks)
                        pb = MB_SIZES[blocks[-1]]
                        s_ps2 = [psS.tile([128, 3, 512], f32, tag="sps",
                                          name=f"sps{e}") for e in range(2)]
                        for i, mb in enumerate(blocks):
                            pbi = MB_SIZES[mb]
                            for e in range(2):
                                nc.tensor.matmul(
                                    s_ps2[e][0:pbi, i, 0:NSUB],
                                    kts[e][bps[e]:bps[e] + 16,
                                           mb * 128:mb * 128 + pbi],
                                    qts[e][bps[e]:bps[e] + 16, nc0:nc0 + NSUB],
                                    start=True, stop=True,
                                    tile_position=(bps[e], 0))
                        p_sb2 = [pP.tile([128, 3, NSUB], f32, tag="psb",
                                        name=f"psb{e}") for e in range(2)]
                        for e in range(2):
                            nc.scalar.activation(
                                out=p_sb2[e][0:pb, 0:gsz, :],
                                in_=s_ps2[e][0:pb, 0:gsz, 0:NSUB], func=Exp)
                        for i, mb in enumerate(blocks):
                            pbi = MB_SIZES[mb]
                            for e in range(2):
                                h = pair[e]
                                nc.tensor.matmul(
                                    o_ps2[e][0:65, 0:NSUB],
                                    vT_sb[0:pbi, mb, h // 4,
                                          65 * (h % 4):65 * (h % 4) + 65],
                                    p_sb2[e][0:pbi, i, :],
                                    start=(mb == 0), stop=(mb == NB - 1))
                    for e in range(2):
                        h = pair[e]
                        o_ps = o_ps2[e]
                        # softmax denominator: row 64 of o_ps
                        idx = h * 2 + c2
                        rsh = pEp.tile([128, NSUB], f32, tag="rsh")
                        nc.vector.tensor_copy(rsh[64:65, :], o_ps[64:65, 0:NSUB])
                        nc.sync.dma_start(
                            out=rsd_d[idx:idx + 1, :], in_=rsh[64:65, :])
                        rb = pEp.tile([64, NSUB], f32, tag="rb")
                        nc.sync.dma_start(
                            out=rb[0:64, :],
                            in_=rsd_d[idx:idx + 1, :].partition_broadcast(64))
                        rbr = pEp.tile([64, NSUB], f32, tag="rbr")
                        scr = pEp.tile([64, NSUB], f32, tag="scr")
                        nc.vector.reciprocal_approx_accurate(
                            out=rbr[:], in_=rb[0:64, :], scratch=scr[:])
                        nc.vector.tensor_mul(
                            out=of_sb[0:64, h, nc0:nc0 + NSUB],
                            in0=o_ps[0:64, 0:NSUB], in1=rbr[:])

        # output projection: out = wp_eff @ [o; 1]
        with tc.tile_pool(name="psY", bufs=2, space="PSUM") as psY:
            for mo in range(2):
                for c2 in range(2):
                    nc0 = c2 * NSUB
                    y_ps = psY.tile([128, 512], f32, tag="yps")
                    for kc in range(9):
                        if kc < 8:
                            lhsT = wp_sb[0:64, kc, mo * 128:(mo + 1) * 128]
                            rhs = of_sb[0:64, kc, nc0:nc0 + NSUB]
                        else:
                            lhsT = wp_sb[0:1, 8, mo * 128:(mo + 1) * 128]
                            rhs = ones_sb[0:1, nc0:nc0 + NSUB]
                        nc.tensor.matmul(
                            y_ps[0:128, 0:NSUB], lhsT, rhs,
                            start=(kc == 0), stop=(kc == 8))
                    nc.vector.tensor_copy(
                        y_sb[:, mo, nc0:nc0 + NSUB], y_ps[0:128, 0:NSUB])
            for mo in range(2):
                nc.sync.dma_start(
                    out=out_d[mo * 128:(mo + 1) * 128, :], in_=y_sb[:, mo, :])

    nc.compile()
    return nc


def get_graph():
    global _GRAPH
    if _GRAPH is None:
        _GRAPH = _build_graph()
    return _GRAPH


def make_in_maps(x, wq, sq, bq, wk, sk, bk, wv, sv, bv, wp, sp, bp):
    f = np.float32
    x2 = np.asarray(x, f).reshape(B, C, N)
    ones_row = np.ones((1, N), f)
    wq = np.asarray(wq, f); sq = np.asarray(sq, f); bq = np.asarray(bq, f)
    wk = np.asarray(wk, f); sk = np.asarray(sk, f); bk = np.asarray(bk, f)
    wv = np.asarray(wv, f); sv = np.asarray(sv, f); bv = np.asarray(bv, f)
    wp = np.asarray(wp, f); sp = np.asarray(sp, f); bp = np.asarray(bp, f)

    wq_eff = np.concatenate([(wq * sq[:, None]).T, bq[None, :]], 0).astype(f)
    wk_eff = np.concatenate([(wk * sk[:, None]).T, bk[None, :]], 0).astype(f)
    wv_base = wv * sv[:, None]  # (512, 256)
    wv_arr = np.zeros((257, 520), f)
    for h in range(NUM_HEADS):
        col = 260 * (h // 4) + 65 * (h % 4)
        wv_arr[0:256, col:col + 64] = wv_base[64 * h:64 * h + 64, :].T
        wv_arr[256, col:col + 64] = bv[64 * h:64 * h + 64]
        wv_arr[256, col + 64] = 1.0
    wp_eff = (wp * sp[:, None]).T.astype(f)  # (512, 256), row c = 64h+d
    wp_arr = np.zeros((64, 9, 256), f)
    wp_arr[:, 0:8, :] = wp_eff.reshape(8, 64, 256).transpose(1, 0, 2)
    wp_arr[0, 8, :] = bp

    in_maps = []
    for core in range(8):
        b, j = core // 4, core % 4
        xa_full = np.ascontiguousarray(np.concatenate([x2[b], ones_row], 0))
        xq_c = np.ascontiguousarray(xa_full[:, j * NCHUNK:(j + 1) * NCHUNK])
        in_maps.append(dict(xa=xa_full, xq=xq_c, wq=wq_eff, wk=wk_eff,
                            wv=wv_arr, wp=wp_arr))
    return in_maps


def assemble_output(results):
    y = np.zeros((B, C, N), np.float32)
    for core in range(8):
        b, j = core // 4, core % 4
        y[b, :, j * NCHUNK:(j + 1) * NCHUNK] = results[core]["out"]
    return y.reshape(B, C, HH, WW)


def kernel(**inputs):
    from concourse.bass_utils import run_bass_kernel_spmd
    nc = get_graph()
    in_maps = make_in_maps(**inputs)
    res = run_bass_kernel_spmd(nc, in_maps, core_ids=list(range(8)))
    return assemble_output(res.results)


if __name__ == "__main__":
    rng = np.random.default_rng(0)
    ins = dict(
        x=rng.standard_normal((2, 256, 56, 56), np.float32),
        wq=rng.standard_normal((128, 256), np.float32) * 0.05,
        sq=rng.random(128, np.float32),
        bq=rng.standard_normal(128, np.float32) * 0.05,
        wk=rng.standard_normal((128, 256), np.float32) * 0.05,
        sk=rng.random(128, np.float32),
        bk=rng.standard_normal(128, np.float32) * 0.05,
        wv=rng.standard_normal((512, 256), np.float32) * 0.05,
        sv=rng.random(512, np.float32),
        bv=rng.standard_normal(512, np.float32) * 0.05,
        wp=rng.standard_normal((256, 512), np.float32) * 0.05,
        sp=rng.random(256, np.float32),
        bp=rng.standard_normal(256, np.float32) * 0.05,
    )
    out = kernel(**ins)
    print("out", out.shape, out.dtype, float(np.abs(out).mean()))


# revision 8
# speedup vs baseline: 1.4822x; 1.3508x over previous
"""Trainium2 Bass kernel for nn_Attention (dense transformer attention block).

Reference computation (per batch b):
  q = BN(wq @ x)  -> (8 heads, 16, 3136)
  k = BN(wk @ x)  -> (8, 16, 3136)
  v = BN(wv @ x)  -> (8, 64, 3136)
  attn = softmax(q^T k)  (scores over 3136x3136 tokens, no scaling)
  o = attn @ v^T -> (8, 64, 3136) -> (512, 56, 56)
  out = BN(wp @ o) -> (256, 56, 56)

Sharding: 8 cores = 2 batches x 4 token-chunks of 784 query tokens.
Each core computes k/v for the full 3136 tokens (cheap) and attention +
output projection for its own 784 query tokens. Zero collectives.

Device algorithm per core (flash-style, f32 end to end):
  - All BN scale factors folded into weights host-side; biases folded via an
    appended ones-row on x (K=257 contraction).
  - S_T[m, n-chunk] = k_blk^T q  (K=16), exp on ACT from PSUM,
    o'[65, n] += v'^T_blk @ exp(S_T_blk)  where v' has an appended ones
    column so row 64 of o' accumulates the softmax denominator.
  - o = o'[0:64] * reciprocal(o'[64]) broadcast via DMA.
  - out = wp_eff @ [o; 1].
"""

import os
import sys

for _p in ("/opt/trn_rl_repo", "/root/.axon_site/_ro/trn_rl_repo"):
    if os.path.isdir(_p) and _p not in sys.path:
        sys.path.insert(0, _p)

import numpy as np

NUM_HEADS = 8
KEY_DIM = 16
D_HEAD = 64
B = 2
C = 256
HH = 56
WW = 56
N = HH * WW          # 3136 tokens
NCHUNK = N // 4      # 784 query tokens per core
NSUB = NCHUNK // 2   # 392, fits one PSUM bank
NB = (N + 127) // 128            # 25 key-blocks
MB_SIZES = [128] * 24 + [64]
KS = [128, 128, 1]               # contraction chunks for K=257
GROUPS = [list(range(g * 3, min(g * 3 + 3, NB))) for g in range(9)]

_GRAPH = None


def _build_graph():
    import concourse.bass as bass  # noqa: F401
    import concourse.mybir as mybir
    import concourse.tile as tile
    from concourse import bacc
    from contextlib import ExitStack

    f32 = mybir.dt.float32
    bf16 = mybir.dt.bfloat16
    Exp = mybir.ActivationFunctionType.Exp

    nc = bacc.Bacc("TRN2", target_bir_lowering=False, debug=False, num_devices=8)
    xa_d = nc.dram_tensor("xa", [257, N], f32, kind="ExternalInput").ap()
    xq_d = nc.dram_tensor("xq", [257, NCHUNK], f32, kind="ExternalInput").ap()
    wq_d = nc.dram_tensor("wq", [257, 128], f32, kind="ExternalInput").ap()
    wk_d = nc.dram_tensor("wk", [257, 128], f32, kind="ExternalInput").ap()
    wv_d = nc.dram_tensor("wv", [257, 520], f32, kind="ExternalInput").ap()
    wp_d = nc.dram_tensor("wp", [64, 9, 256], f32, kind="ExternalInput").ap()
    out_d = nc.dram_tensor("out", [256, NCHUNK], f32, kind="ExternalOutput").ap()
    rsd_d = nc.dram_tensor("rsd", [16, NSUB], f32).ap()  # rowsum bounce

    with tile.TileContext(nc) as tc, ExitStack() as stk:
        const = stk.enter_context(tc.tile_pool(name="const", bufs=1))
        xq_sb = const.tile([128, 3, NCHUNK], f32, tag="xq")
        wq_sb = const.tile([128, 3, 128], f32, tag="wq")
        wk_sb = const.tile([128, 3, 128], f32, tag="wk")
        wv_sb = const.tile([128, 3, 520], f32, tag="wv")
        wp_sb = const.tile([64, 9, 256], f32, tag="wp")
        ones_sb = const.tile([1, NCHUNK], f32, tag="ones")
        # per-head 32-aligned base partitions: head h -> (k_lo if h<4 else
        # k_hi) partitions [32*(h%4), 32*(h%4)+16)
        k_lo = const.tile([128, N], bf16, tag="klo")
        k_hi = const.tile([128, N], bf16, tag="khi")
        q_lo = const.tile([128, NCHUNK], bf16, tag="qlo")
        q_hi = const.tile([128, NCHUNK], bf16, tag="qhi")
        # v'^T: [m-in-block, block, head-half, 65*hh + (64 v cols + ones col)]
        vT_sb = const.tile([128, NB, 2, 260], bf16, tag="vt")
        of_sb = const.tile([64, 8, NCHUNK], f32, tag="of")
        y_sb = const.tile([128, 2, NCHUNK], f32, tag="y")

        for kc in range(3):
            ks, off = KS[kc], 128 * kc
            nc.sync.dma_start(out=wq_sb[0:ks, kc, :], in_=wq_d[off:off + ks, :])
            nc.sync.dma_start(out=wk_sb[0:ks, kc, :], in_=wk_d[off:off + ks, :])
            nc.sync.dma_start(out=wv_sb[0:ks, kc, :], in_=wv_d[off:off + ks, :])
            nc.sync.dma_start(out=xq_sb[0:ks, kc, :], in_=xq_d[off:off + ks, :])
        nc.sync.dma_start(out=wp_sb[:], in_=wp_d[:])
        nc.vector.memset(ones_sb[:], 1.0)

        with tc.tile_pool(name="stageA", bufs=2) as sA, \
             tc.tile_pool(name="psA", bufs=2, space="PSUM") as psA, \
             tc.tile_pool(name="tmpA", bufs=1) as tA:
            k_sb = tA.tile([128, N], bf16, tag="ksb")
            q_sb = tA.tile([128, NCHUNK], bf16, tag="qsb")
            # q projection (bias folded via ones row of xq)
            for c2 in range(2):
                q_ps = psA.tile([128, 512], f32, tag="qkps")
                for kc in range(3):
                    nc.tensor.matmul(
                        q_ps[0:128, 0:NSUB],
                        wq_sb[0:KS[kc], kc, :],
                        xq_sb[0:KS[kc], kc, c2 * NSUB:(c2 + 1) * NSUB],
                        start=(kc == 0), stop=(kc == 2))
                nc.vector.tensor_copy(
                    q_sb[:, c2 * NSUB:(c2 + 1) * NSUB], q_ps[0:128, 0:NSUB])
            # k projection + v'^T, in 512-column passes over xa
            for p in range(7):
                c0 = 512 * p
                cw = min(512, N - c0)
                xa_t = sA.tile([128, 3, 512], f32, tag="xat")
                for kc in range(3):
                    nc.sync.dma_start(
                        out=xa_t[0:KS[kc], kc, 0:cw],
                        in_=xa_d[128 * kc:128 * kc + KS[kc], c0:c0 + cw])
                k_ps = psA.tile([128, 512], f32, tag="qkps")
                for kc in range(3):
                    nc.tensor.matmul(
                        k_ps[0:128, 0:cw],
                        wk_sb[0:KS[kc], kc, :],
                        xa_t[0:KS[kc], kc, 0:cw],
                        start=(kc == 0), stop=(kc == 2))
                nc.vector.tensor_copy(k_sb[:, c0:c0 + cw], k_ps[0:128, 0:cw])
                for mbi in range(4):
                    mb = 4 * p + mbi
                    if mb >= NB:
                        break
                    pb = MB_SIZES[mb]
                    vt_ps = psA.tile([128, 2, 512], f32, tag="vtps")
                    for half in range(2):
                        for kc in range(3):
                            nc.tensor.matmul(
                                vt_ps[0:pb, half, 0:260],
                                xa_t[0:KS[kc], kc, mbi * 128:mbi * 128 + pb],
                                wv_sb[0:KS[kc], kc, half * 260:(half + 1) * 260],
                                start=(kc == 0), stop=(kc == 2))
                    nc.vector.tensor_copy(
                        vT_sb[0:pb, mb, :, :], vt_ps[0:pb, :, 0:260])
            # regroup heads onto 32-aligned bases
            for h in range(8):
                kt = k_lo if h < 4 else k_hi
                qt = q_lo if h < 4 else q_hi
                bp_ = 32 * (h % 4)
                nc.sync.dma_start(out=kt[bp_:bp_ + 16, :], in_=k_sb[16 * h:16 * h + 16, :])
                nc.sync.dma_start(out=qt[bp_:bp_ + 16, :], in_=q_sb[16 * h:16 * h + 16, :])

        # main attention loop — head PAIRS with different PE row groups
        # interleaved so the PE never stalls on one head's exp and LDWEIGHTS
        # can pull ahead (different row_grp).
        PAIRS = [(0, 2), (1, 3), (4, 6), (5, 7)]
        with tc.tile_pool(name="pP", bufs=4) as pP, \
             tc.tile_pool(name="pEp", bufs=3) as pEp, \
             tc.tile_pool(name="psS", bufs=2, space="PSUM") as psS, \
             tc.tile_pool(name="psO", bufs=2, space="PSUM") as psO:
            for pair in PAIRS:
                kts = [k_lo if h < 4 else k_hi for h in pair]
                qts = [q_lo if h < 4 else q_hi for h in pair]
                bps = [32 * (h % 4) for h in pair]
                for c2 in range(2):
                    nc0 = c2 * NSUB
                    o_ps2 = [psO.tile([128, 512], f32, tag="ops", name=f"ops{e}") for e in range(2)]
                    for blocks in GROUPS:
                        gsz = len(blocks)
                        pb = MB_SIZES[blocks[-1]]
                        s_ps2 = [psS.tile([128, 3, 512], f32, tag="sps",
                                          name=f"sps{e}") for e in range(2)]
                        for i, mb in enumerate(blocks):
                            pbi = MB_SIZES[mb]
                            for e in range(2):
                                nc.tensor.matmul(
                                    s_ps2[e][0:pbi, i, 0:NSUB],
                                    kts[e][bps[e]:bps[e] + 16,
                                           mb * 128:mb * 128 + pbi],
                                    qts[e][bps[e]:bps[e] + 16, nc0:nc0 + NSUB],
                                    start=True, stop=True,
                                    tile_position=(bps[e], 0))
                        p_sb2 = [pP.tile([128, 3, NSUB], bf16, tag="psb",
                                        name=f"psb{e}") for e in range(2)]
                        for e in range(2):
                            nc.scalar.activation(
                                out=p_sb2[e][0:pb, 0:gsz, :],
                                in_=s_ps2[e][0:pb, 0:gsz, 0:NSUB], func=Exp)
                        for i, mb in enumerate(blocks):
                            pbi = MB_SIZES[mb]
                            for e in range(2):
                                h = pair[e]
                                nc.tensor.matmul(
                                    o_ps2[e][0:65, 0:NSUB],
                                    vT_sb[0:pbi, mb, h // 4,
                                          65 * (h % 4):65 * (h % 4) + 65],
                                    p_sb2[e][0:pbi, i, :],
                                    start=(mb == 0), stop=(mb == NB - 1))
                    for e in range(2):
                        h = pair[e]
                        o_ps = o_ps2[e]
                        # softmax denominator: row 64 of o_ps
                        idx = h * 2 + c2
                        rsh = pEp.tile([128, NSUB], f32, tag="rsh")
                        nc.vector.tensor_copy(rsh[64:65, :], o_ps[64:65, 0:NSUB])
                        nc.sync.dma_start(
                            out=rsd_d[idx:idx + 1, :], in_=rsh[64:65, :])
                        rb = pEp.tile([64, NSUB], f32, tag="rb")
                        nc.sync.dma_start(
                            out=rb[0:64, :],
                            in_=rsd_d[idx:idx + 1, :].partition_broadcast(64))
                        rbr = pEp.tile([64, NSUB], f32, tag="rbr")
                        scr = pEp.tile([64, NSUB], f32, tag="scr")
                        nc.vector.reciprocal_approx_accurate(
                            out=rbr[:], in_=rb[0:64, :], scratch=scr[:])
                        nc.vector.tensor_mul(
                            out=of_sb[0:64, h, nc0:nc0 + NSUB],
                            in0=o_ps[0:64, 0:NSUB], in1=rbr[:])

        # output projection: out = wp_eff @ [o; 1]
        with tc.tile_pool(name="psY", bufs=2, space="PSUM") as psY:
            for mo in range(2):
                for c2 in range(2):
                    nc0 = c2 * NSUB
                    y_ps = psY.tile([128, 512], f32, tag="yps")
                    for kc in range(9):
                        if kc < 8:
                            lhsT = wp_sb[0:64, kc, mo * 128:(mo + 1) * 128]
                            rhs = of_sb[0:64, kc, nc0:nc0 + NSUB]
                        else:
                            lhsT = wp_sb[0:1, 8, mo * 128:(mo + 1) * 128]
                            rhs = ones_sb[0:1, nc0:nc0 + NSUB]
                        nc.tensor.matmul(
                            y_ps[0:128, 0:NSUB], lhsT, rhs,
                            start=(kc == 0), stop=(kc == 8))
                    nc.vector.tensor_copy(
                        y_sb[:, mo, nc0:nc0 + NSUB], y_ps[0:128, 0:NSUB])
            for mo in range(2):
                nc.sync.dma_start(
                    out=out_d[mo * 128:(mo + 1) * 128, :], in_=y_sb[:, mo, :])

    nc.compile()
    return nc


def get_graph():
    global _GRAPH
    if _GRAPH is None:
        _GRAPH = _build_graph()
    return _GRAPH


def make_in_maps(x, wq, sq, bq, wk, sk, bk, wv, sv, bv, wp, sp, bp):
    f = np.float32
    x2 = np.asarray(x, f).reshape(B, C, N)
    ones_row = np.ones((1, N), f)
    wq = np.asarray(wq, f); sq = np.asarray(sq, f); bq = np.asarray(bq, f)
    wk = np.asarray(wk, f); sk = np.asarray(sk, f); bk = np.asarray(bk, f)
    wv = np.asarray(wv, f); sv = np.asarray(sv, f); bv = np.asarray(bv, f)
    wp = np.asarray(wp, f); sp = np.asarray(sp, f); bp = np.asarray(bp, f)

    wq_eff = np.concatenate([(wq * sq[:, None]).T, bq[None, :]], 0).astype(f)
    wk_eff = np.concatenate([(wk * sk[:, None]).T, bk[None, :]], 0).astype(f)
    wv_base = wv * sv[:, None]  # (512, 256)
    wv_arr = np.zeros((257, 520), f)
    for h in range(NUM_HEADS):
        col = 260 * (h // 4) + 65 * (h % 4)
        wv_arr[0:256, col:col + 64] = wv_base[64 * h:64 * h + 64, :].T
        wv_arr[256, col:col + 64] = bv[64 * h:64 * h + 64]
        wv_arr[256, col + 64] = 1.0
    wp_eff = (wp * sp[:, None]).T.astype(f)  # (512, 256), row c = 64h+d
    wp_arr = np.zeros((64, 9, 256), f)
    wp_arr[:, 0:8, :] = wp_eff.reshape(8, 64, 256).transpose(1, 0, 2)
    wp_arr[0, 8, :] = bp

    in_maps = []
    for core in range(8):
        b, j = core // 4, core % 4
        xa_full = np.ascontiguousarray(np.concatenate([x2[b], ones_row], 0))
        xq_c = np.ascontiguousarray(xa_full[:, j * NCHUNK:(j + 1) * NCHUNK])
        in_maps.append(dict(xa=xa_full, xq=xq_c, wq=wq_eff, wk=wk_eff,
                            wv=wv_arr, wp=wp_arr))
    return in_maps


def assemble_output(results):
    y = np.zeros((B, C, N), np.float32)
    for core in range(8):
        b, j = core // 4, core % 4
        y[b, :, j * NCHUNK:(j + 1) * NCHUNK] = results[core]["out"]
    return y.reshape(B, C, HH, WW)


def kernel(**inputs):
    from concourse.bass_utils import run_bass_kernel_spmd
    nc = get_graph()
    in_maps = make_in_maps(**inputs)
    res = run_bass_kernel_spmd(nc, in_maps, core_ids=list(range(8)))
    return assemble_output(res.results)


if __name__ == "__main__":
    rng = np.random.default_rng(0)
    ins = dict(
        x=rng.standard_normal((2, 256, 56, 56), np.float32),
        wq=rng.standard_normal((128, 256), np.float32) * 0.05,
        sq=rng.random(128, np.float32),
        bq=rng.standard_normal(128, np.float32) * 0.05,
        wk=rng.standard_normal((128, 256), np.float32) * 0.05,
        sk=rng.random(128, np.float32),
        bk=rng.standard_normal(128, np.float32) * 0.05,
        wv=rng.standard_normal((512, 256), np.float32) * 0.05,
        sv=rng.random(512, np.float32),
        bv=rng.standard_normal(512, np.float32) * 0.05,
        wp=rng.standard_normal((256, 512), np.float32) * 0.05,
        sp=rng.random(256, np.float32),
        bp=rng.standard_normal(256, np.float32) * 0.05,
    )
    out = kernel(**ins)
    print("out", out.shape, out.dtype, float(np.abs(out).mean()))


# revision 9
# speedup vs baseline: 2.2397x; 1.5111x over previous
"""Trainium2 Bass kernel for nn_Attention (dense transformer attention block).

Reference computation (per batch b):
  q = BN(wq @ x)  -> (8 heads, 16, 3136)
  k = BN(wk @ x)  -> (8, 16, 3136)
  v = BN(wv @ x)  -> (8, 64, 3136)
  attn = softmax(q^T k)  (scores over 3136x3136 tokens, no scaling)
  o = attn @ v^T -> (8, 64, 3136) -> (512, 56, 56)
  out = BN(wp @ o) -> (256, 56, 56)

Sharding: 8 cores = 2 batches x 4 token-chunks of 784 query tokens.
Each core computes k/v for the full 3136 tokens (cheap) and attention +
output projection for its own 784 query tokens. Zero collectives.

Device algorithm per core (flash-style, f32 end to end):
  - All BN scale factors folded into weights host-side; biases folded via an
    appended ones-row on x (K=257 contraction).
  - S_T[m, n-chunk] = k_blk^T q  (K=16), exp on ACT from PSUM,
    o'[65, n] += v'^T_blk @ exp(S_T_blk)  where v' has an appended ones
    column so row 64 of o' accumulates the softmax denominator.
  - o = o'[0:64] * reciprocal(o'[64]) broadcast via DMA.
  - out = wp_eff @ [o; 1].
"""

import os
import sys

for _p in ("/opt/trn_rl_repo", "/root/.axon_site/_ro/trn_rl_repo"):
    if os.path.isdir(_p) and _p not in sys.path:
        sys.path.insert(0, _p)

import numpy as np

NUM_HEADS = 8
KEY_DIM = 16
D_HEAD = 64
B = 2
C = 256
HH = 56
WW = 56
N = HH * WW          # 3136 tokens
NCHUNK = N // 4      # 784 query tokens per core
NSUB = NCHUNK // 2   # 392, fits one PSUM bank
NB = (N + 127) // 128            # 25 key-blocks
MB_SIZES = [128] * 24 + [64]
KS = [128, 128, 1]               # contraction chunks for K=257
GROUPS = [list(range(g * 3, min(g * 3 + 3, NB))) for g in range(9)]

_GRAPH = None


def _build_graph():
    import concourse.bass as bass  # noqa: F401
    import concourse.mybir as mybir
    import concourse.tile as tile
    from concourse import bacc
    from contextlib import ExitStack

    f32 = mybir.dt.float32
    bf16 = mybir.dt.bfloat16
    Exp = mybir.ActivationFunctionType.Exp

    nc = bacc.Bacc("TRN2", target_bir_lowering=False, debug=False, num_devices=8)
    xa_d = nc.dram_tensor("xa", [257, N], bf16, kind="ExternalInput").ap()
    xq_d = nc.dram_tensor("xq", [257, NCHUNK], bf16, kind="ExternalInput").ap()
    wq_d = nc.dram_tensor("wq", [257, 128], bf16, kind="ExternalInput").ap()
    wk_d = nc.dram_tensor("wk", [257, 128], bf16, kind="ExternalInput").ap()
    wv_d = nc.dram_tensor("wv", [257, 520], bf16, kind="ExternalInput").ap()
    wp_d = nc.dram_tensor("wp", [64, 9, 256], f32, kind="ExternalInput").ap()
    out_d = nc.dram_tensor("out", [256, NCHUNK], f32, kind="ExternalOutput").ap()
    rsd_d = nc.dram_tensor("rsd", [16, NSUB], f32).ap()  # rowsum bounce

    with tile.TileContext(nc) as tc, ExitStack() as stk:
        const = stk.enter_context(tc.tile_pool(name="const", bufs=1))
        xq_sb = const.tile([128, 3, NCHUNK], bf16, tag="xq")
        wq_sb = const.tile([128, 3, 128], bf16, tag="wq")
        wk_sb = const.tile([128, 3, 128], bf16, tag="wk")
        wv_sb = const.tile([128, 3, 520], bf16, tag="wv")
        wp_sb = const.tile([64, 9, 256], f32, tag="wp")
        ones_sb = const.tile([1, NCHUNK], f32, tag="ones")
        # per-head 32-aligned base partitions: head h -> (k_lo if h<4 else
        # k_hi) partitions [32*(h%4), 32*(h%4)+16)
        k_lo = const.tile([128, N], bf16, tag="klo")
        k_hi = const.tile([128, N], bf16, tag="khi")
        q_lo = const.tile([128, NCHUNK], bf16, tag="qlo")
        q_hi = const.tile([128, NCHUNK], bf16, tag="qhi")
        # v'^T: [m-in-block, block, head-half, 65*hh + (64 v cols + ones col)]
        vT_sb = const.tile([128, NB, 2, 260], bf16, tag="vt")
        of_sb = const.tile([64, 8, NCHUNK], f32, tag="of")
        y_sb = const.tile([128, 2, NCHUNK], f32, tag="y")

        for kc in range(3):
            ks, off = KS[kc], 128 * kc
            nc.sync.dma_start(out=wq_sb[0:ks, kc, :], in_=wq_d[off:off + ks, :])
            nc.sync.dma_start(out=wk_sb[0:ks, kc, :], in_=wk_d[off:off + ks, :])
            nc.sync.dma_start(out=wv_sb[0:ks, kc, :], in_=wv_d[off:off + ks, :])
            nc.sync.dma_start(out=xq_sb[0:ks, kc, :], in_=xq_d[off:off + ks, :])
        nc.sync.dma_start(out=wp_sb[:], in_=wp_d[:])
        nc.vector.memset(ones_sb[:], 1.0)

        with tc.tile_pool(name="stageA", bufs=2) as sA, \
             tc.tile_pool(name="psA", bufs=2, space="PSUM") as psA, \
             tc.tile_pool(name="tmpA", bufs=1) as tA:
            k_sb = tA.tile([128, N], bf16, tag="ksb")
            q_sb = tA.tile([128, NCHUNK], bf16, tag="qsb")
            # q projection (bias folded via ones row of xq)
            for c2 in range(2):
                q_ps = psA.tile([128, 512], f32, tag="qkps")
                for kc in range(3):
                    nc.tensor.matmul(
                        q_ps[0:128, 0:NSUB],
                        wq_sb[0:KS[kc], kc, :],
                        xq_sb[0:KS[kc], kc, c2 * NSUB:(c2 + 1) * NSUB],
                        start=(kc == 0), stop=(kc == 2))
                nc.vector.tensor_copy(
                    q_sb[:, c2 * NSUB:(c2 + 1) * NSUB], q_ps[0:128, 0:NSUB])
            # k projection + v'^T, in 512-column passes over xa
            for p in range(7):
                c0 = 512 * p
                cw = min(512, N - c0)
                xa_t = sA.tile([128, 3, 512], bf16, tag="xat")
                for kc in range(3):
                    nc.sync.dma_start(
                        out=xa_t[0:KS[kc], kc, 0:cw],
                        in_=xa_d[128 * kc:128 * kc + KS[kc], c0:c0 + cw])
                k_ps = psA.tile([128, 512], f32, tag="qkps")
                for kc in range(3):
                    nc.tensor.matmul(
                        k_ps[0:128, 0:cw],
                        wk_sb[0:KS[kc], kc, :],
                        xa_t[0:KS[kc], kc, 0:cw],
                        start=(kc == 0), stop=(kc == 2))
                nc.vector.tensor_copy(k_sb[:, c0:c0 + cw], k_ps[0:128, 0:cw])
                for mbi in range(4):
                    mb = 4 * p + mbi
                    if mb >= NB:
                        break
                    pb = MB_SIZES[mb]
                    vt_ps = psA.tile([128, 2, 512], f32, tag="vtps")
                    for half in range(2):
                        for kc in range(3):
                            nc.tensor.matmul(
                                vt_ps[0:pb, half, 0:260],
                                xa_t[0:KS[kc], kc, mbi * 128:mbi * 128 + pb],
                                wv_sb[0:KS[kc], kc, half * 260:(half + 1) * 260],
                                start=(kc == 0), stop=(kc == 2))
                    nc.vector.tensor_copy(
                        vT_sb[0:pb, mb, :, :], vt_ps[0:pb, :, 0:260])
            # regroup heads onto 32-aligned bases
            for h in range(8):
                kt = k_lo if h < 4 else k_hi
                qt = q_lo if h < 4 else q_hi
                bp_ = 32 * (h % 4)
                nc.sync.dma_start(out=kt[bp_:bp_ + 16, :], in_=k_sb[16 * h:16 * h + 16, :])
                nc.sync.dma_start(out=qt[bp_:bp_ + 16, :], in_=q_sb[16 * h:16 * h + 16, :])

        # main attention loop — head PAIRS with different PE row groups
        # interleaved so the PE never stalls on one head's exp and LDWEIGHTS
        # can pull ahead (different row_grp).
        PAIRS = [(0, 2), (1, 3), (4, 6), (5, 7)]
        with tc.tile_pool(name="pP", bufs=4) as pP, \
             tc.tile_pool(name="pEp", bufs=3) as pEp, \
             tc.tile_pool(name="psS", bufs=2, space="PSUM") as psS, \
             tc.tile_pool(name="psO", bufs=2, space="PSUM") as psO:
            for pair in PAIRS:
                kts = [k_lo if h < 4 else k_hi for h in pair]
                qts = [q_lo if h < 4 else q_hi for h in pair]
                bps = [32 * (h % 4) for h in pair]
                for c2 in range(2):
                    nc0 = c2 * NSUB
                    o_ps2 = [psO.tile([128, 512], f32, tag="ops", name=f"ops{e}") for e in range(2)]
                    for blocks in GROUPS:
                        gsz = len(blocks)
                        pb = MB_SIZES[blocks[-1]]
                        s_ps2 = [psS.tile([128, 3, 512], f32, tag="sps",
                                          name=f"sps{e}") for e in range(2)]
                        for i, mb in enumerate(blocks):
                            pbi = MB_SIZES[mb]
                            for e in range(2):
                                nc.tensor.matmul(
                                    s_ps2[e][0:pbi, i, 0:NSUB],
                                    kts[e][bps[e]:bps[e] + 16,
                                           mb * 128:mb * 128 + pbi],
                                    qts[e][bps[e]:bps[e] + 16, nc0:nc0 + NSUB],
                                    start=True, stop=True,
                                    tile_position=(bps[e], 0))
                        p_sb2 = [pP.tile([128, 3, NSUB], bf16, tag="psb",
                                        name=f"psb{e}") for e in range(2)]
                        for e in range(2):
                            nc.scalar.activation(
                                out=p_sb2[e][0:pb, 0:gsz, :],
                                in_=s_ps2[e][0:pb, 0:gsz, 0:NSUB], func=Exp)
                        for i, mb in enumerate(blocks):
                            pbi = MB_SIZES[mb]
                            for e in range(2):
                                h = pair[e]
                                nc.tensor.matmul(
                                    o_ps2[e][0:65, 0:NSUB],
                                    vT_sb[0:pbi, mb, h // 4,
                                          65 * (h % 4):65 * (h % 4) + 65],
                                    p_sb2[e][0:pbi, i, :],
                                    start=(mb == 0), stop=(mb == NB - 1))
                    for e in range(2):
                        h = pair[e]
                        o_ps = o_ps2[e]
                        # softmax denominator: row 64 of o_ps
                        idx = h * 2 + c2
                        rsh = pEp.tile([128, NSUB], f32, tag="rsh")
                        nc.vector.tensor_copy(rsh[64:65, :], o_ps[64:65, 0:NSUB])
                        nc.sync.dma_start(
                            out=rsd_d[idx:idx + 1, :], in_=rsh[64:65, :])
                        rb = pEp.tile([64, NSUB], f32, tag="rb")
                        nc.sync.dma_start(
                            out=rb[0:64, :],
                            in_=rsd_d[idx:idx + 1, :].partition_broadcast(64))
                        rbr = pEp.tile([64, NSUB], f32, tag="rbr")
                        scr = pEp.tile([64, NSUB], f32, tag="scr")
                        nc.vector.reciprocal_approx_accurate(
                            out=rbr[:], in_=rb[0:64, :], scratch=scr[:])
                        nc.vector.tensor_mul(
                            out=of_sb[0:64, h, nc0:nc0 + NSUB],
                            in0=o_ps[0:64, 0:NSUB], in1=rbr[:])

        # output projection: out = wp_eff @ [o; 1]
        with tc.tile_pool(name="psY", bufs=2, space="PSUM") as psY:
            for mo in range(2):
                for c2 in range(2):
                    nc0 = c2 * NSUB
                    y_ps = psY.tile([128, 512], f32, tag="yps")
                    for kc in range(9):
                        if kc < 8:
                            lhsT = wp_sb[0:64, kc, mo * 128:(mo + 1) * 128]
                            rhs = of_sb[0:64, kc, nc0:nc0 + NSUB]
                        else:
                            lhsT = wp_sb[0:1, 8, mo * 128:(mo + 1) * 128]
                            rhs = ones_sb[0:1, nc0:nc0 + NSUB]
                        nc.tensor.matmul(
                            y_ps[0:128, 0:NSUB], lhsT, rhs,
                            start=(kc == 0), stop=(kc == 8))
                    nc.vector.tensor_copy(
                        y_sb[:, mo, nc0:nc0 + NSUB], y_ps[0:128, 0:NSUB])
            for mo in range(2):
                nc.sync.dma_start(
                    out=out_d[mo * 128:(mo + 1) * 128, :], in_=y_sb[:, mo, :])

    nc.compile()
    return nc


def get_graph():
    global _GRAPH
    if _GRAPH is None:
        _GRAPH = _build_graph()
    return _GRAPH


def make_in_maps(x, wq, sq, bq, wk, sk, bk, wv, sv, bv, wp, sp, bp):
    import ml_dtypes
    bf = ml_dtypes.bfloat16
    f = np.float32
    x2 = np.asarray(x, f).reshape(B, C, N)
    ones_row = np.ones((1, N), f)
    wq = np.asarray(wq, f); sq = np.asarray(sq, f); bq = np.asarray(bq, f)
    wk = np.asarray(wk, f); sk = np.asarray(sk, f); bk = np.asarray(bk, f)
    wv = np.asarray(wv, f); sv = np.asarray(sv, f); bv = np.asarray(bv, f)
    wp = np.asarray(wp, f); sp = np.asarray(sp, f); bp = np.asarray(bp, f)

    wq_eff = np.concatenate([(wq * sq[:, None]).T, bq[None, :]], 0).astype(f)
    wk_eff = np.concatenate([(wk * sk[:, None]).T, bk[None, :]], 0).astype(f)
    wv_base = wv * sv[:, None]  # (512, 256)
    wv_arr = np.zeros((257, 520), f)
    for h in range(NUM_HEADS):
        col = 260 * (h // 4) + 65 * (h % 4)
        wv_arr[0:256, col:col + 64] = wv_base[64 * h:64 * h + 64, :].T
        wv_arr[256, col:col + 64] = bv[64 * h:64 * h + 64]
        wv_arr[256, col + 64] = 1.0
    wp_eff = (wp * sp[:, None]).T.astype(f)  # (512, 256), row c = 64h+d
    wp_arr = np.zeros((64, 9, 256), f)
    wp_arr[:, 0:8, :] = wp_eff.reshape(8, 64, 256).transpose(1, 0, 2)
    wp_arr[0, 8, :] = bp

    in_maps = []
    for core in range(8):
        b, j = core // 4, core % 4
        xa_full = np.ascontiguousarray(np.concatenate([x2[b], ones_row], 0))
        xq_c = np.ascontiguousarray(xa_full[:, j * NCHUNK:(j + 1) * NCHUNK])
        in_maps.append(dict(xa=xa_full.astype(bf), xq=xq_c.astype(bf),
                            wq=wq_eff.astype(bf), wk=wk_eff.astype(bf),
                            wv=wv_arr.astype(bf), wp=wp_arr))
    return in_maps


def assemble_output(results):
    y = np.zeros((B, C, N), np.float32)
    for core in range(8):
        b, j = core // 4, core % 4
        y[b, :, j * NCHUNK:(j + 1) * NCHUNK] = results[core]["out"]
    return y.reshape(B, C, HH, WW)


def kernel(**inputs):
    from concourse.bass_utils import run_bass_kernel_spmd
    nc = get_graph()
    in_maps = make_in_maps(**inputs)
    res = run_bass_kernel_spmd(nc, in_maps, core_ids=list(range(8)))
    return assemble_output(res.results)


if __name__ == "__main__":
    rng = np.random.default_rng(0)
    ins = dict(
        x=rng.standard_normal((2, 256, 56, 56), np.float32),
        wq=rng.standard_normal((128, 256), np.float32) * 0.05,
        sq=rng.random(128, np.float32),
        bq=rng.standard_normal(128, np.float32) * 0.05,
        wk=rng.standard_normal((128, 256), np.float32) * 0.05,
        sk=rng.random(128, np.float32),
        bk=rng.standard_normal(128, np.float32) * 0.05,
        wv=rng.standard_normal((512, 256), np.float32) * 0.05,
        sv=rng.random(512, np.float32),
        bv=rng.standard_normal(512, np.float32) * 0.05,
        wp=rng.standard_normal((256, 512), np.float32) * 0.05,
        sp=rng.random(256, np.float32),
        bp=rng.standard_normal(256, np.float32) * 0.05,
    )
    out = kernel(**ins)
    print("out", out.shape, out.dtype, float(np.abs(out).mean()))


# revision 10
# speedup vs baseline: 2.6341x; 1.1761x over previous
"""Trainium2 Bass kernel for nn_Attention (dense transformer attention block).

Reference computation (per batch b):
  q = BN(wq @ x)  -> (8 heads, 16, 3136)
  k = BN(wk @ x)  -> (8, 16, 3136)
  v = BN(wv @ x)  -> (8, 64, 3136)
  attn = softmax(q^T k)  (scores over 3136x3136 tokens, no scaling)
  o = attn @ v^T -> (8, 64, 3136) -> (512, 56, 56)
  out = BN(wp @ o) -> (256, 56, 56)

Sharding: 8 cores = 2 batches x 4 token-chunks of 784 query tokens.
Each core computes k/v for the full 3136 tokens (cheap) and attention +
output projection for its own 784 query tokens. Zero collectives.

Device algorithm per core (flash-style, f32 end to end):
  - All BN scale factors folded into weights host-side; biases folded via an
    appended ones-row on x (K=257 contraction).
  - S_T[m, n-chunk] = k_blk^T q  (K=16), exp on ACT from PSUM,
    o'[65, n] += v'^T_blk @ exp(S_T_blk)  where v' has an appended ones
    column so row 64 of o' accumulates the softmax denominator.
  - o = o'[0:64] * reciprocal(o'[64]) broadcast via DMA.
  - out = wp_eff @ [o; 1].
"""

import os
import sys

for _p in ("/opt/trn_rl_repo", "/root/.axon_site/_ro/trn_rl_repo"):
    if os.path.isdir(_p) and _p not in sys.path:
        sys.path.insert(0, _p)

import numpy as np

NUM_HEADS = 8
KEY_DIM = 16
D_HEAD = 64
B = 2
C = 256
HH = 56
WW = 56
N = HH * WW          # 3136 tokens
NCHUNK = N // 4      # 784 query tokens per core
NSUB = NCHUNK // 2   # 392, fits one PSUM bank
NB = (N + 127) // 128            # 25 key-blocks
MB_SIZES = [128] * 24 + [64]
KS = [128, 128, 1]               # contraction chunks for K=257
GROUPS = [list(range(g * 3, min(g * 3 + 3, NB))) for g in range(9)]

_GRAPH = None


def _build_graph():
    import concourse.bass as bass  # noqa: F401
    import concourse.mybir as mybir
    import concourse.tile as tile
    from concourse import bacc
    from contextlib import ExitStack

    f32 = mybir.dt.float32
    bf16 = mybir.dt.bfloat16
    Exp = mybir.ActivationFunctionType.Exp

    nc = bacc.Bacc("TRN2", target_bir_lowering=False, debug=False, num_devices=8)
    xa_d = nc.dram_tensor("xa", [257, N], bf16, kind="ExternalInput").ap()
    xq_d = nc.dram_tensor("xq", [257, NCHUNK], bf16, kind="ExternalInput").ap()
    wq_d = nc.dram_tensor("wq", [257, 128], bf16, kind="ExternalInput").ap()
    wk_d = nc.dram_tensor("wk", [257, 128], bf16, kind="ExternalInput").ap()
    wv_d = nc.dram_tensor("wv", [257, 520], bf16, kind="ExternalInput").ap()
    wp_d = nc.dram_tensor("wp", [64, 9, 256], bf16, kind="ExternalInput").ap()
    out_d = nc.dram_tensor("out", [256, NCHUNK], f32, kind="ExternalOutput").ap()
    rsd_d = nc.dram_tensor("rsd", [16, NSUB], f32).ap()  # rowsum bounce

    with tile.TileContext(nc) as tc, ExitStack() as stk:
        const = stk.enter_context(tc.tile_pool(name="const", bufs=1))
        xq_sb = const.tile([128, 3, NCHUNK], bf16, tag="xq")
        wq_sb = const.tile([128, 3, 128], bf16, tag="wq")
        wk_sb = const.tile([128, 3, 128], bf16, tag="wk")
        wv_sb = const.tile([128, 3, 520], bf16, tag="wv")
        wp_sb = const.tile([64, 9, 256], bf16, tag="wp")
        ones_sb = const.tile([1, NCHUNK], bf16, tag="ones")
        # per-head 32-aligned base partitions: head h -> (k_lo if h<4 else
        # k_hi) partitions [32*(h%4), 32*(h%4)+16)
        k_lo = const.tile([128, N], bf16, tag="klo")
        k_hi = const.tile([128, N], bf16, tag="khi")
        q_lo = const.tile([128, NCHUNK], bf16, tag="qlo")
        q_hi = const.tile([128, NCHUNK], bf16, tag="qhi")
        # replicas shifted by +32 partitions so consecutive blocks of one head
        # use different PE row groups (4-way concurrent scores)
        k_lo2 = const.tile([128, N], bf16, tag="klo2")
        k_hi2 = const.tile([128, N], bf16, tag="khi2")
        q_lo2 = const.tile([128, NCHUNK], bf16, tag="qlo2")
        q_hi2 = const.tile([128, NCHUNK], bf16, tag="qhi2")
        # v'^T: [m-in-block, block, head-half, 65*hh + (64 v cols + ones col)]
        vT_sb = const.tile([128, NB, 2, 260], bf16, tag="vt")
        of_sb = const.tile([64, 8, NCHUNK], bf16, tag="of")
        y_sb = const.tile([128, 2, NCHUNK], f32, tag="y")

        for kc in range(3):
            ks, off = KS[kc], 128 * kc
            nc.sync.dma_start(out=wq_sb[0:ks, kc, :], in_=wq_d[off:off + ks, :])
            nc.sync.dma_start(out=wk_sb[0:ks, kc, :], in_=wk_d[off:off + ks, :])
            nc.sync.dma_start(out=wv_sb[0:ks, kc, :], in_=wv_d[off:off + ks, :])
            nc.sync.dma_start(out=xq_sb[0:ks, kc, :], in_=xq_d[off:off + ks, :])
        nc.sync.dma_start(out=wp_sb[:], in_=wp_d[:])
        nc.vector.memset(ones_sb[:], 1.0)

        with tc.tile_pool(name="stageA", bufs=2) as sA, \
             tc.tile_pool(name="psA", bufs=2, space="PSUM") as psA, \
             tc.tile_pool(name="psAV", bufs=3, space="PSUM") as psAV, \
             tc.tile_pool(name="tmpA", bufs=1) as tA:
            k_sb = tA.tile([128, N], bf16, tag="ksb")
            q_sb = tA.tile([128, NCHUNK], bf16, tag="qsb")
            # q projection (bias folded via ones row of xq)
            for c2 in range(2):
                q_ps = psA.tile([128, 512], f32, tag="qkps")
                for kc in range(3):
                    nc.tensor.matmul(
                        q_ps[0:128, 0:NSUB],
                        wq_sb[0:KS[kc], kc, :],
                        xq_sb[0:KS[kc], kc, c2 * NSUB:(c2 + 1) * NSUB],
                        start=(kc == 0), stop=(kc == 2))
                nc.vector.tensor_copy(
                    q_sb[:, c2 * NSUB:(c2 + 1) * NSUB], q_ps[0:128, 0:NSUB])
            # k projection + v'^T, in 512-column passes over xa
            for p in range(7):
                c0 = 512 * p
                cw = min(512, N - c0)
                xa_t = sA.tile([128, 3, 512], bf16, tag="xat")
                for kc in range(3):
                    nc.sync.dma_start(
                        out=xa_t[0:KS[kc], kc, 0:cw],
                        in_=xa_d[128 * kc:128 * kc + KS[kc], c0:c0 + cw])
                k_ps = psA.tile([128, 512], f32, tag="qkps")
                for kc in range(3):
                    nc.tensor.matmul(
                        k_ps[0:128, 0:cw],
                        wk_sb[0:KS[kc], kc, :],
                        xa_t[0:KS[kc], kc, 0:cw],
                        start=(kc == 0), stop=(kc == 2))
                if p % 2 == 0:
                    nc.vector.tensor_copy(k_sb[:, c0:c0 + cw], k_ps[0:128, 0:cw])
                else:
                    nc.scalar.copy(k_sb[:, c0:c0 + cw], k_ps[0:128, 0:cw])
                for mbi in range(4):
                    mb = 4 * p + mbi
                    if mb >= NB:
                        break
                    pb = MB_SIZES[mb]
                    vt_ps = psAV.tile([128, 2, 512], f32, tag="vtps")
                    for half in range(2):
                        for kc in range(3):
                            nc.tensor.matmul(
                                vt_ps[0:pb, half, 0:260],
                                xa_t[0:KS[kc], kc, mbi * 128:mbi * 128 + pb],
                                wv_sb[0:KS[kc], kc, half * 260:(half + 1) * 260],
                                start=(kc == 0), stop=(kc == 2))
                    if mb % 2 == 0:
                        nc.vector.tensor_copy(
                            vT_sb[0:pb, mb, :, :], vt_ps[0:pb, :, 0:260])
                    else:
                        nc.scalar.copy(
                            vT_sb[0:pb, mb, :, :], vt_ps[0:pb, :, 0:260])
            # regroup heads onto 32-aligned bases
            for h in range(8):
                kt = k_lo if h < 4 else k_hi
                qt = q_lo if h < 4 else q_hi
                kt2 = k_lo2 if h < 4 else k_hi2
                qt2 = q_lo2 if h < 4 else q_hi2
                bp_ = 32 * (h % 4)
                bp2 = (bp_ + 32) % 128
                nc.sync.dma_start(out=kt[bp_:bp_ + 16, :], in_=k_sb[16 * h:16 * h + 16, :])
                nc.sync.dma_start(out=qt[bp_:bp_ + 16, :], in_=q_sb[16 * h:16 * h + 16, :])
                nc.sync.dma_start(out=kt2[bp2:bp2 + 16, :], in_=k_sb[16 * h:16 * h + 16, :])
                nc.sync.dma_start(out=qt2[bp2:bp2 + 16, :], in_=q_sb[16 * h:16 * h + 16, :])

        # main attention loop — head PAIRS with different PE row groups
        # interleaved so the PE never stalls on one head's exp and LDWEIGHTS
        # can pull ahead (different row_grp).
        PAIRS = [(0, 2), (1, 3), (4, 6), (5, 7)]
        with tc.tile_pool(name="pP", bufs=4) as pP, \
             tc.tile_pool(name="pEp", bufs=3) as pEp, \
             tc.tile_pool(name="psS", bufs=2, space="PSUM") as psS, \
             tc.tile_pool(name="psO", bufs=2, space="PSUM") as psO:
            for pair in PAIRS:
                kts = [k_lo if h < 4 else k_hi for h in pair]
                qts = [q_lo if h < 4 else q_hi for h in pair]
                kts2 = [k_lo2 if h < 4 else k_hi2 for h in pair]
                qts2 = [q_lo2 if h < 4 else q_hi2 for h in pair]
                bps = [32 * (h % 4) for h in pair]
                bps2 = [(32 * (h % 4) + 32) % 128 for h in pair]
                for c2 in range(2):
                    nc0 = c2 * NSUB
                    o_ps2 = [psO.tile([128, 512], f32, tag="ops", name=f"ops{e}") for e in range(2)]
                    for blocks in GROUPS:
                        gsz = len(blocks)
                        pb = MB_SIZES[blocks[-1]]
                        s_ps2 = [psS.tile([128, 3, 512], f32, tag="sps",
                                          name=f"sps{e}") for e in range(2)]
                        for i, mb in enumerate(blocks):
                            pbi = MB_SIZES[mb]
                            for e in range(2):
                                if mb % 2 == 0:
                                    kte, qte, be = kts[e], qts[e], bps[e]
                                else:
                                    kte, qte, be = kts2[e], qts2[e], bps2[e]
                                nc.tensor.matmul(
                                    s_ps2[e][0:pbi, i, 0:NSUB],
                                    kte[be:be + 16,
                                        mb * 128:mb * 128 + pbi],
                                    qte[be:be + 16, nc0:nc0 + NSUB],
                                    start=True, stop=True,
                                    tile_position=(be, 0))
                        p_sb2 = [pP.tile([128, 3, NSUB], bf16, tag="psb",
                                        name=f"psb{e}") for e in range(2)]
                        for e in range(2):
                            nc.scalar.activation(
                                out=p_sb2[e][0:pb, 0:gsz, :],
                                in_=s_ps2[e][0:pb, 0:gsz, 0:NSUB], func=Exp)
                        for i, mb in enumerate(blocks):
                            pbi = MB_SIZES[mb]
                            for e in range(2):
                                h = pair[e]
                                nc.tensor.matmul(
                                    o_ps2[e][0:65, 0:NSUB],
                                    vT_sb[0:pbi, mb, h // 4,
                                          65 * (h % 4):65 * (h % 4) + 65],
                                    p_sb2[e][0:pbi, i, :],
                                    start=(mb == 0), stop=(mb == NB - 1))
                    for e in range(2):
                        h = pair[e]
                        o_ps = o_ps2[e]
                        # softmax denominator: row 64 of o_ps
                        idx = h * 2 + c2
                        rsh = pEp.tile([128, NSUB], f32, tag="rsh")
                        nc.vector.tensor_copy(rsh[64:65, :], o_ps[64:65, 0:NSUB])
                        nc.sync.dma_start(
                            out=rsd_d[idx:idx + 1, :], in_=rsh[64:65, :])
                        rb = pEp.tile([64, NSUB], f32, tag="rb")
                        nc.sync.dma_start(
                            out=rb[0:64, :],
                            in_=rsd_d[idx:idx + 1, :].partition_broadcast(64))
                        rbr = pEp.tile([64, NSUB], f32, tag="rbr")
                        scr = pEp.tile([64, NSUB], f32, tag="scr")
                        nc.vector.reciprocal_approx_accurate(
                            out=rbr[:], in_=rb[0:64, :], scratch=scr[:])
                        nc.vector.tensor_mul(
                            out=of_sb[0:64, h, nc0:nc0 + NSUB],
                            in0=o_ps[0:64, 0:NSUB], in1=rbr[:])

        # output projection: out = wp_eff @ [o; 1]
        with tc.tile_pool(name="psY", bufs=2, space="PSUM") as psY:
            for mo in range(2):
                for c2 in range(2):
                    nc0 = c2 * NSUB
                    y_ps = psY.tile([128, 512], f32, tag="yps")
                    for kc in range(9):
                        if kc < 8:
                            lhsT = wp_sb[0:64, kc, mo * 128:(mo + 1) * 128]
                            rhs = of_sb[0:64, kc, nc0:nc0 + NSUB]
                        else:
                            lhsT = wp_sb[0:1, 8, mo * 128:(mo + 1) * 128]
                            rhs = ones_sb[0:1, nc0:nc0 + NSUB]
                        nc.tensor.matmul(
                            y_ps[0:128, 0:NSUB], lhsT, rhs,
                            start=(kc == 0), stop=(kc == 8))
                    nc.vector.tensor_copy(
                        y_sb[:, mo, nc0:nc0 + NSUB], y_ps[0:128, 0:NSUB])
            for mo in range(2):
                nc.sync.dma_start(
                    out=out_d[mo * 128:(mo + 1) * 128, :], in_=y_sb[:, mo, :])

    nc.compile()
    return nc


def get_graph():
    global _GRAPH
    if _GRAPH is None:
        _GRAPH = _build_graph()
    return _GRAPH


def make_in_maps(x, wq, sq, bq, wk, sk, bk, wv, sv, bv, wp, sp, bp):
    import ml_dtypes
    bf = ml_dtypes.bfloat16
    f = np.float32
    x2 = np.asarray(x, f).reshape(B, C, N)
    ones_row = np.ones((1, N), f)
    wq = np.asarray(wq, f); sq = np.asarray(sq, f); bq = np.asarray(bq, f)
    wk = np.asarray(wk, f); sk = np.asarray(sk, f); bk = np.asarray(bk, f)
    wv = np.asarray(wv, f); sv = np.asarray(sv, f); bv = np.asarray(bv, f)
    wp = np.asarray(wp, f); sp = np.asarray(sp, f); bp = np.asarray(bp, f)

    wq_eff = np.concatenate([(wq * sq[:, None]).T, bq[None, :]], 0).astype(f)
    wk_eff = np.concatenate([(wk * sk[:, None]).T, bk[None, :]], 0).astype(f)
    wv_base = wv * sv[:, None]  # (512, 256)
    wv_arr = np.zeros((257, 520), f)
    for h in range(NUM_HEADS):
        col = 260 * (h // 4) + 65 * (h % 4)
        wv_arr[0:256, col:col + 64] = wv_base[64 * h:64 * h + 64, :].T
        wv_arr[256, col:col + 64] = bv[64 * h:64 * h + 64]
        wv_arr[256, col + 64] = 1.0
    wp_eff = (wp * sp[:, None]).T.astype(f)  # (512, 256), row c = 64h+d
    wp_arr = np.zeros((64, 9, 256), f)
    wp_arr[:, 0:8, :] = wp_eff.reshape(8, 64, 256).transpose(1, 0, 2)
    wp_arr[0, 8, :] = bp

    in_maps = []
    for core in range(8):
        b, j = core // 4, core % 4
        xa_full = np.ascontiguousarray(np.concatenate([x2[b], ones_row], 0))
        xq_c = np.ascontiguousarray(xa_full[:, j * NCHUNK:(j + 1) * NCHUNK])
        in_maps.append(dict(xa=xa_full.astype(bf), xq=xq_c.astype(bf),
                            wq=wq_eff.astype(bf), wk=wk_eff.astype(bf),
                            wv=wv_arr.astype(bf), wp=wp_arr.astype(bf)))
    return in_maps


def assemble_output(results):
    y = np.zeros((B, C, N), np.float32)
    for core in range(8):
        b, j = core // 4, core % 4
        y[b, :, j * NCHUNK:(j + 1) * NCHUNK] = results[core]["out"]
    return y.reshape(B, C, HH, WW)


def kernel(**inputs):
    from concourse.bass_utils import run_bass_kernel_spmd
    nc = get_graph()
    in_maps = make_in_maps(**inputs)
    res = run_bass_kernel_spmd(nc, in_maps, core_ids=list(range(8)))
    return assemble_output(res.results)


if __name__ == "__main__":
    rng = np.random.default_rng(0)
    ins = dict(
        x=rng.standard_normal((2, 256, 56, 56), np.float32),
        wq=rng.standard_normal((128, 256), np.float32) * 0.05,
        sq=rng.random(128, np.float32),
        bq=rng.standard_normal(128, np.float32) * 0.05,
        wk=rng.standard_normal((128, 256), np.float32) * 0.05,
        sk=rng.random(128, np.float32),
        bk=rng.standard_normal(128, np.float32) * 0.05,
        wv=rng.standard_normal((512, 256), np.float32) * 0.05,
        sv=rng.random(512, np.float32),
        bv=rng.standard_normal(512, np.float32) * 0.05,
        wp=rng.standard_normal((256, 512), np.float32) * 0.05,
        sp=rng.random(256, np.float32),
        bp=rng.standard_normal(256, np.float32) * 0.05,
    )
    out = kernel(**ins)
    print("out", out.shape, out.dtype, float(np.abs(out).mean()))


# revision 11
# speedup vs baseline: 2.6549x; 1.0079x over previous
"""Trainium2 Bass kernel for nn_Attention (dense transformer attention block).

Reference computation (per batch b):
  q = BN(wq @ x)  -> (8 heads, 16, 3136)
  k = BN(wk @ x)  -> (8, 16, 3136)
  v = BN(wv @ x)  -> (8, 64, 3136)
  attn = softmax(q^T k)  (scores over 3136x3136 tokens, no scaling)
  o = attn @ v^T -> (8, 64, 3136) -> (512, 56, 56)
  out = BN(wp @ o) -> (256, 56, 56)

Sharding: 8 cores = 2 batches x 4 token-chunks of 784 query tokens.
Each core computes k/v for the full 3136 tokens (cheap) and attention +
output projection for its own 784 query tokens. Zero collectives.

Device algorithm per core (flash-style, f32 end to end):
  - All BN scale factors folded into weights host-side; biases folded via an
    appended ones-row on x (K=257 contraction).
  - S_T[m, n-chunk] = k_blk^T q  (K=16), exp on ACT from PSUM,
    o'[65, n] += v'^T_blk @ exp(S_T_blk)  where v' has an appended ones
    column so row 64 of o' accumulates the softmax denominator.
  - o = o'[0:64] * reciprocal(o'[64]) broadcast via DMA.
  - out = wp_eff @ [o; 1].
"""

import os
import sys

for _p in ("/opt/trn_rl_repo", "/root/.axon_site/_ro/trn_rl_repo"):
    if os.path.isdir(_p) and _p not in sys.path:
        sys.path.insert(0, _p)

import numpy as np

NUM_HEADS = 8
KEY_DIM = 16
D_HEAD = 64
B = 2
C = 256
HH = 56
WW = 56
N = HH * WW          # 3136 tokens
NCHUNK = N // 4      # 784 query tokens per core
NSUB = NCHUNK // 2   # 392, fits one PSUM bank
NB = (N + 127) // 128            # 25 key-blocks
MB_SIZES = [128] * 24 + [64]
KS = [128, 128, 1]               # contraction chunks for K=257
GROUPS = [list(range(g * 3, min(g * 3 + 3, NB))) for g in range(9)]

_GRAPH = None


def _build_graph():
    import concourse.bass as bass  # noqa: F401
    import concourse.mybir as mybir
    import concourse.tile as tile
    from concourse import bacc
    from contextlib import ExitStack

    f32 = mybir.dt.float32
    bf16 = mybir.dt.bfloat16
    Exp = mybir.ActivationFunctionType.Exp

    nc = bacc.Bacc("TRN2", target_bir_lowering=False, debug=False, num_devices=8)
    xa_d = nc.dram_tensor("xa", [257, N], bf16, kind="ExternalInput").ap()
    xq_d = nc.dram_tensor("xq", [257, NCHUNK], bf16, kind="ExternalInput").ap()
    wq_d = nc.dram_tensor("wq", [257, 128], bf16, kind="ExternalInput").ap()
    wk_d = nc.dram_tensor("wk", [257, 128], bf16, kind="ExternalInput").ap()
    wv_d = nc.dram_tensor("wv", [257, 520], bf16, kind="ExternalInput").ap()
    wp_d = nc.dram_tensor("wp", [64, 9, 256], bf16, kind="ExternalInput").ap()
    out_d = nc.dram_tensor("out", [256, NCHUNK], f32, kind="ExternalOutput").ap()
    rsd_d = nc.dram_tensor("rsd", [16, NSUB], f32).ap()  # rowsum bounce

    with tile.TileContext(nc) as tc, ExitStack() as stk:
        const = stk.enter_context(tc.tile_pool(name="const", bufs=1))
        xq_sb = const.tile([128, 3, NCHUNK], bf16, tag="xq")
        wq_sb = const.tile([128, 3, 128], bf16, tag="wq")
        wk_sb = const.tile([128, 3, 128], bf16, tag="wk")
        wv_sb = const.tile([128, 3, 520], bf16, tag="wv")
        wp_sb = const.tile([64, 9, 256], bf16, tag="wp")
        ones_sb = const.tile([1, NCHUNK], bf16, tag="ones")
        # per-head 32-aligned base partitions: head h -> (k_lo if h<4 else
        # k_hi) partitions [32*(h%4), 32*(h%4)+16)
        k_lo = const.tile([128, N], bf16, tag="klo")
        k_hi = const.tile([128, N], bf16, tag="khi")
        q_lo = const.tile([128, NCHUNK], bf16, tag="qlo")
        q_hi = const.tile([128, NCHUNK], bf16, tag="qhi")
        # replicas shifted by +32 partitions so consecutive blocks of one head
        # use different PE row groups (4-way concurrent scores)
        k_lo2 = const.tile([128, N], bf16, tag="klo2")
        k_hi2 = const.tile([128, N], bf16, tag="khi2")
        q_lo2 = const.tile([128, NCHUNK], bf16, tag="qlo2")
        q_hi2 = const.tile([128, NCHUNK], bf16, tag="qhi2")
        # v'^T: [m-in-block, block, head-half, 65*hh + (64 v cols + ones col)]
        vT_sb = const.tile([128, NB, 2, 260], bf16, tag="vt")
        of_sb = const.tile([64, 8, NCHUNK], bf16, tag="of")
        y_sb = const.tile([128, 2, NCHUNK], f32, tag="y")

        for kc in range(3):
            ks, off = KS[kc], 128 * kc
            nc.sync.dma_start(out=wq_sb[0:ks, kc, :], in_=wq_d[off:off + ks, :])
            nc.sync.dma_start(out=wk_sb[0:ks, kc, :], in_=wk_d[off:off + ks, :])
            nc.sync.dma_start(out=wv_sb[0:ks, kc, :], in_=wv_d[off:off + ks, :])
            nc.sync.dma_start(out=xq_sb[0:ks, kc, :], in_=xq_d[off:off + ks, :])
        nc.sync.dma_start(out=wp_sb[:], in_=wp_d[:])
        nc.vector.memset(ones_sb[:], 1.0)

        with tc.tile_pool(name="stageA", bufs=2) as sA, \
             tc.tile_pool(name="psA", bufs=2, space="PSUM") as psA, \
             tc.tile_pool(name="psAV", bufs=3, space="PSUM") as psAV, \
             tc.tile_pool(name="tmpA", bufs=1) as tA:
            k_sb = tA.tile([128, N], bf16, tag="ksb")
            q_sb = tA.tile([128, NCHUNK], bf16, tag="qsb")
            # q projection (bias folded via ones row of xq)
            for c2 in range(2):
                q_ps = psA.tile([128, 512], f32, tag="qkps")
                for kc in range(3):
                    nc.tensor.matmul(
                        q_ps[0:128, 0:NSUB],
                        wq_sb[0:KS[kc], kc, :],
                        xq_sb[0:KS[kc], kc, c2 * NSUB:(c2 + 1) * NSUB],
                        start=(kc == 0), stop=(kc == 2))
                nc.vector.tensor_copy(
                    q_sb[:, c2 * NSUB:(c2 + 1) * NSUB], q_ps[0:128, 0:NSUB])
            # k projection + v'^T, in 512-column passes over xa
            for p in range(7):
                c0 = 512 * p
                cw = min(512, N - c0)
                xa_t = sA.tile([128, 3, 512], bf16, tag="xat")
                for kc in range(3):
                    nc.sync.dma_start(
                        out=xa_t[0:KS[kc], kc, 0:cw],
                        in_=xa_d[128 * kc:128 * kc + KS[kc], c0:c0 + cw])
                k_ps = psA.tile([128, 512], f32, tag="qkps")
                for kc in range(3):
                    nc.tensor.matmul(
                        k_ps[0:128, 0:cw],
                        wk_sb[0:KS[kc], kc, :],
                        xa_t[0:KS[kc], kc, 0:cw],
                        start=(kc == 0), stop=(kc == 2))
                if p % 2 == 0:
                    nc.vector.tensor_copy(k_sb[:, c0:c0 + cw], k_ps[0:128, 0:cw])
                else:
                    nc.scalar.copy(k_sb[:, c0:c0 + cw], k_ps[0:128, 0:cw])
                for mbi in range(4):
                    mb = 4 * p + mbi
                    if mb >= NB:
                        break
                    pb = MB_SIZES[mb]
                    vt_ps = psAV.tile([128, 2, 512], f32, tag="vtps")
                    for half in range(2):
                        for kc in range(3):
                            nc.tensor.matmul(
                                vt_ps[0:pb, half, 0:260],
                                xa_t[0:KS[kc], kc, mbi * 128:mbi * 128 + pb],
                                wv_sb[0:KS[kc], kc, half * 260:(half + 1) * 260],
                                start=(kc == 0), stop=(kc == 2))
                    if mb % 2 == 0:
                        nc.vector.tensor_copy(
                            vT_sb[0:pb, mb, :, :], vt_ps[0:pb, :, 0:260])
                    else:
                        nc.scalar.copy(
                            vT_sb[0:pb, mb, :, :], vt_ps[0:pb, :, 0:260])
            # regroup heads onto 32-aligned bases
            for h in range(8):
                kt = k_lo if h < 4 else k_hi
                qt = q_lo if h < 4 else q_hi
                kt2 = k_lo2 if h < 4 else k_hi2
                qt2 = q_lo2 if h < 4 else q_hi2
                bp_ = 32 * (h % 4)
                bp2 = (bp_ + 32) % 128
                nc.sync.dma_start(out=kt[bp_:bp_ + 16, :], in_=k_sb[16 * h:16 * h + 16, :])
                nc.sync.dma_start(out=qt[bp_:bp_ + 16, :], in_=q_sb[16 * h:16 * h + 16, :])
                nc.sync.dma_start(out=kt2[bp2:bp2 + 16, :], in_=k_sb[16 * h:16 * h + 16, :])
                nc.sync.dma_start(out=qt2[bp2:bp2 + 16, :], in_=q_sb[16 * h:16 * h + 16, :])

        # main attention loop, software-pipelined:
        # iteration i = (head-pair, n-chunk). During iteration i's scores+exp
        # phase, the PE executes iteration i-1's o'-accumulation matmuls as
        # filler, so it never idles waiting on ACT (keeps HAM warm).
        # PSUM: scores 2 slots x 3 banks + o' 2 slots x 1 bank = 8 banks.
        PAIRS = [(0, 2), (1, 3), (4, 6), (5, 7)]
        ITERS = [(pair, c2) for pair in PAIRS for c2 in range(2)]

        def emit_scores_group(pair, c2, blocks, s_ps2, kts, qts, kts2, qts2,
                              bps, bps2):
            nc0 = c2 * NSUB
            for i, mb in enumerate(blocks):
                pbi = MB_SIZES[mb]
                for e in range(2):
                    if mb % 2 == 0:
                        kte, qte, be = kts[e], qts[e], bps[e]
                    else:
                        kte, qte, be = kts2[e], qts2[e], bps2[e]
                    nc.tensor.matmul(
                        s_ps2[e][0:pbi, i, 0:NSUB],
                        kte[be:be + 16, mb * 128:mb * 128 + pbi],
                        qte[be:be + 16, nc0:nc0 + NSUB],
                        start=True, stop=True,
                        tile_position=(be, 0))

        def emit_filler(job):
            # one o'-accumulation matmul of the previous iteration
            (pair, c2, e, p_tile, i, mb, o_ps2) = job
            h = pair[e]
            pbi = MB_SIZES[mb]
            nc.tensor.matmul(
                o_ps2[e][0:65, 0:NSUB],
                vT_sb[0:pbi, mb, h // 4, 65 * (h % 4):65 * (h % 4) + 65],
                p_tile[0:pbi, i, :],
                start=(mb == 0), stop=(mb == NB - 1))

        def emit_epilogue(pair, c2, o_ps2):
            nc0 = c2 * NSUB
            for e in range(2):
                h = pair[e]
                o_ps = o_ps2[e]
                idx = h * 2 + c2
                rsh = pEp.tile([128, NSUB], f32, tag="rsh")
                nc.vector.tensor_copy(rsh[64:65, :], o_ps[64:65, 0:NSUB])
                nc.sync.dma_start(out=rsd_d[idx:idx + 1, :], in_=rsh[64:65, :])
                rb = pEp.tile([64, NSUB], f32, tag="rb")
                nc.sync.dma_start(
                    out=rb[0:64, :],
                    in_=rsd_d[idx:idx + 1, :].partition_broadcast(64))
                rbr = pEp.tile([64, NSUB], f32, tag="rbr")
                scr = pEp.tile([64, NSUB], f32, tag="scr")
                nc.vector.reciprocal_approx_accurate(
                    out=rbr[:], in_=rb[0:64, :], scratch=scr[:])
                nc.vector.tensor_mul(
                    out=of_sb[0:64, h, nc0:nc0 + NSUB],
                    in0=o_ps[0:64, 0:NSUB], in1=rbr[:])

        with tc.tile_pool(name="pP", bufs=22) as pP, \
             tc.tile_pool(name="pEp", bufs=3) as pEp, \
             tc.tile_pool(name="psS", bufs=2, space="PSUM") as psS, \
             tc.tile_pool(name="psO", bufs=2, space="PSUM") as psO:
            prev = None  # (pair, c2, p_tiles) of the previous iteration
            for it in range(len(ITERS) + 1):
                cur = ITERS[it] if it < len(ITERS) else None
                fillers = []
                if prev is not None:
                    ppair, pc2, p_tiles = prev
                    o_ps2 = [psO.tile([128, 512], f32, tag="ops",
                                      name=f"ops{e}") for e in range(2)]
                    for mb in range(NB):
                        g, i = mb // 3, mb % 3
                        for e in range(2):
                            fillers.append((ppair, pc2, e, p_tiles[g][e],
                                            i, mb, o_ps2))
                if cur is None:
                    for job in fillers:
                        emit_filler(job)
                    emit_epilogue(ppair, pc2, o_ps2)
                    break
                pair, c2 = cur
                kts = [k_lo if h < 4 else k_hi for h in pair]
                qts = [q_lo if h < 4 else q_hi for h in pair]
                kts2 = [k_lo2 if h < 4 else k_hi2 for h in pair]
                qts2 = [q_lo2 if h < 4 else q_hi2 for h in pair]
                bps = [32 * (h % 4) for h in pair]
                bps2 = [(32 * (h % 4) + 32) % 128 for h in pair]
                p_tiles = []
                nfill = len(fillers)
                for g, blocks in enumerate(GROUPS):
                    gsz = len(blocks)
                    pb = MB_SIZES[blocks[-1]]
                    s_ps2 = [psS.tile([128, 3, 512], f32, tag="sps",
                                      name=f"sps{e}") for e in range(2)]
                    emit_scores_group(pair, c2, blocks, s_ps2, kts, qts,
                                      kts2, qts2, bps, bps2)
                    p_sb2 = [pP.tile([128, 3, NSUB], bf16, tag="psb",
                                     name=f"psb{e}") for e in range(2)]
                    for e in range(2):
                        nc.scalar.activation(
                            out=p_sb2[e][0:pb, 0:gsz, :],
                            in_=s_ps2[e][0:pb, 0:gsz, 0:NSUB], func=Exp)
                    p_tiles.append(p_sb2)
                    # interleave previous iteration's o' matmuls as PE filler
                    lo = nfill * g // len(GROUPS)
                    hi = nfill * (g + 1) // len(GROUPS)
                    for job in fillers[lo:hi]:
                        emit_filler(job)
                if prev is not None:
                    emit_epilogue(ppair, pc2, o_ps2)
                prev = (pair, c2, p_tiles)

        # output projection: out = wp_eff @ [o; 1]
        with tc.tile_pool(name="psY", bufs=2, space="PSUM") as psY:
            for mo in range(2):
                for c2 in range(2):
                    nc0 = c2 * NSUB
                    y_ps = psY.tile([128, 512], f32, tag="yps")
                    for kc in range(9):
                        if kc < 8:
                            lhsT = wp_sb[0:64, kc, mo * 128:(mo + 1) * 128]
                            rhs = of_sb[0:64, kc, nc0:nc0 + NSUB]
                        else:
                            lhsT = wp_sb[0:1, 8, mo * 128:(mo + 1) * 128]
                            rhs = ones_sb[0:1, nc0:nc0 + NSUB]
                        nc.tensor.matmul(
                            y_ps[0:128, 0:NSUB], lhsT, rhs,
                            start=(kc == 0), stop=(kc == 8))
                    nc.vector.tensor_copy(
                        y_sb[:, mo, nc0:nc0 + NSUB], y_ps[0:128, 0:NSUB])
            for mo in range(2):
                nc.sync.dma_start(
                    out=out_d[mo * 128:(mo + 1) * 128, :], in_=y_sb[:, mo, :])

    nc.compile()
    return nc


def get_graph():
    global _GRAPH
    if _GRAPH is None:
        _GRAPH = _build_graph()
    return _GRAPH


def make_in_maps(x, wq, sq, bq, wk, sk, bk, wv, sv, bv, wp, sp, bp):
    import ml_dtypes
    bf = ml_dtypes.bfloat16
    f = np.float32
    x2 = np.asarray(x, f).reshape(B, C, N)
    ones_row = np.ones((1, N), f)
    wq = np.asarray(wq, f); sq = np.asarray(sq, f); bq = np.asarray(bq, f)
    wk = np.asarray(wk, f); sk = np.asarray(sk, f); bk = np.asarray(bk, f)
    wv = np.asarray(wv, f); sv = np.asarray(sv, f); bv = np.asarray(bv, f)
    wp = np.asarray(wp, f); sp = np.asarray(sp, f); bp = np.asarray(bp, f)

    wq_eff = np.concatenate([(wq * sq[:, None]).T, bq[None, :]], 0).astype(f)
    wk_eff = np.concatenate([(wk * sk[:, None]).T, bk[None, :]], 0).astype(f)
    wv_base = wv * sv[:, None]  # (512, 256)
    wv_arr = np.zeros((257, 520), f)
    for h in range(NUM_HEADS):
        col = 260 * (h // 4) + 65 * (h % 4)
        wv_arr[0:256, col:col + 64] = wv_base[64 * h:64 * h + 64, :].T
        wv_arr[256, col:col + 64] = bv[64 * h:64 * h + 64]
        wv_arr[256, col + 64] = 1.0
    wp_eff = (wp * sp[:, None]).T.astype(f)  # (512, 256), row c = 64h+d
    wp_arr = np.zeros((64, 9, 256), f)
    wp_arr[:, 0:8, :] = wp_eff.reshape(8, 64, 256).transpose(1, 0, 2)
    wp_arr[0, 8, :] = bp

    in_maps = []
    for core in range(8):
        b, j = core // 4, core % 4
        xa_full = np.ascontiguousarray(np.concatenate([x2[b], ones_row], 0))
        xq_c = np.ascontiguousarray(xa_full[:, j * NCHUNK:(j + 1) * NCHUNK])
        in_maps.append(dict(xa=xa_full.astype(bf), xq=xq_c.astype(bf),
                            wq=wq_eff.astype(bf), wk=wk_eff.astype(bf),
                            wv=wv_arr.astype(bf), wp=wp_arr.astype(bf)))
    return in_maps


def assemble_output(results):
    y = np.zeros((B, C, N), np.float32)
    for core in range(8):
        b, j = core // 4, core % 4
        y[b, :, j * NCHUNK:(j + 1) * NCHUNK] = results[core]["out"]
    return y.reshape(B, C, HH, WW)


def kernel(**inputs):
    from concourse.bass_utils import run_bass_kernel_spmd
    nc = get_graph()
    in_maps = make_in_maps(**inputs)
    res = run_bass_kernel_spmd(nc, in_maps, core_ids=list(range(8)))
    return assemble_output(res.results)


if __name__ == "__main__":
    rng = np.random.default_rng(0)
    ins = dict(
        x=rng.standard_normal((2, 256, 56, 56), np.float32),
        wq=rng.standard_normal((128, 256), np.float32) * 0.05,
        sq=rng.random(128, np.float32),
        bq=rng.standard_normal(128, np.float32) * 0.05,
        wk=rng.standard_normal((128, 256), np.float32) * 0.05,
        sk=rng.random(128, np.float32),
        bk=rng.standard_normal(128, np.float32) * 0.05,
        wv=rng.standard_normal((512, 256), np.float32) * 0.05,
        sv=rng.random(512, np.float32),
        bv=rng.standard_normal(512, np.float32) * 0.05,
        wp=rng.standard_normal((256, 512), np.float32) * 0.05,
        sp=rng.random(256, np.float32),
        bp=rng.standard_normal(256, np.float32) * 0.05,
    )
    out = kernel(**ins)
    print("out", out.shape, out.dtype, float(np.abs(out).mean()))


# revision 12
# speedup vs baseline: 2.8528x; 1.0746x over previous
"""Trainium2 Bass kernel for nn_Attention (dense transformer attention block).

Reference computation (per batch b):
  q = BN(wq @ x)  -> (8 heads, 16, 3136)
  k = BN(wk @ x)  -> (8, 16, 3136)
  v = BN(wv @ x)  -> (8, 64, 3136)
  attn = softmax(q^T k)  (scores over 3136x3136 tokens, no scaling)
  o = attn @ v^T -> (8, 64, 3136) -> (512, 56, 56)
  out = BN(wp @ o) -> (256, 56, 56)

Sharding: 8 cores = 2 batches x 4 token-chunks of 784 query tokens.
Each core computes k/v for the full 3136 tokens (cheap) and attention +
output projection for its own 784 query tokens. Zero collectives.

Device algorithm per core (flash-style, f32 end to end):
  - All BN scale factors folded into weights host-side; biases folded via an
    appended ones-row on x (K=257 contraction).
  - S_T[m, n-chunk] = k_blk^T q  (K=16), exp on ACT from PSUM,
    o'[65, n] += v'^T_blk @ exp(S_T_blk)  where v' has an appended ones
    column so row 64 of o' accumulates the softmax denominator.
  - o = o'[0:64] * reciprocal(o'[64]) broadcast via DMA.
  - out = wp_eff @ [o; 1].
"""

import os
import sys

for _p in ("/opt/trn_rl_repo", "/root/.axon_site/_ro/trn_rl_repo"):
    if os.path.isdir(_p) and _p not in sys.path:
        sys.path.insert(0, _p)

import numpy as np

NUM_HEADS = 8
KEY_DIM = 16
D_HEAD = 64
B = 2
C = 256
HH = 56
WW = 56
N = HH * WW          # 3136 tokens
NCHUNK = N // 4      # 784 query tokens per core
NSUB = NCHUNK // 2   # 392, fits one PSUM bank
NB = (N + 127) // 128            # 25 key-blocks
MB_SIZES = [128] * 24 + [64]
KS = [128, 128]                  # contraction chunks for K=256
GROUPS = [list(range(g * 3, min(g * 3 + 3, NB))) for g in range(9)]

_GRAPH = None


def _build_graph():
    import concourse.bass as bass  # noqa: F401
    import concourse.mybir as mybir
    import concourse.tile as tile
    from concourse import bacc
    from contextlib import ExitStack

    f32 = mybir.dt.float32
    bf16 = mybir.dt.bfloat16
    Exp = mybir.ActivationFunctionType.Exp

    nc = bacc.Bacc("TRN2", target_bir_lowering=False, debug=False, num_devices=8)
    xa_d = nc.dram_tensor("xa", [256, N], bf16, kind="ExternalInput").ap()
    xq_d = nc.dram_tensor("xq", [256, NCHUNK], bf16, kind="ExternalInput").ap()
    wq_d = nc.dram_tensor("wq", [256, 128], bf16, kind="ExternalInput").ap()
    wk_d = nc.dram_tensor("wk", [256, 128], bf16, kind="ExternalInput").ap()
    wv_d = nc.dram_tensor("wv", [256, 520], bf16, kind="ExternalInput").ap()
    qb_d = nc.dram_tensor("qb", [128, 1], f32, kind="ExternalInput").ap()
    kb_d = nc.dram_tensor("kb", [128, 1], f32, kind="ExternalInput").ap()
    vb_d = nc.dram_tensor("vb", [1, 520], bf16, kind="ExternalInput").ap()
    pb_d = nc.dram_tensor("pb", [128, 2], f32, kind="ExternalInput").ap()
    wp_d = nc.dram_tensor("wp", [64, 8, 256], bf16, kind="ExternalInput").ap()
    out_d = nc.dram_tensor("out", [256, NCHUNK], f32, kind="ExternalOutput").ap()
    rsd_d = nc.dram_tensor("rsd", [16, NSUB], f32).ap()  # rowsum bounce

    with tile.TileContext(nc) as tc, ExitStack() as stk:
        const = stk.enter_context(tc.tile_pool(name="const", bufs=1))
        xq_sb = const.tile([128, 2, NCHUNK], bf16, tag="xq")
        wq_sb = const.tile([128, 2, 128], bf16, tag="wq")
        wk_sb = const.tile([128, 2, 128], bf16, tag="wk")
        wv_sb = const.tile([128, 2, 520], bf16, tag="wv")
        wp_sb = const.tile([64, 8, 256], bf16, tag="wp")
        qb_sb = const.tile([128, 1], f32, tag="qb")
        kb_sb = const.tile([128, 1], f32, tag="kb")
        vb_sb = const.tile([128, 2, 260], bf16, tag="vb")
        pb_sb = const.tile([128, 2], f32, tag="pb")
        # per-head 32-aligned base partitions: head h -> (k_lo if h<4 else
        # k_hi) partitions [32*(h%4), 32*(h%4)+16)
        k_lo = const.tile([128, N], bf16, tag="klo")
        k_hi = const.tile([128, N], bf16, tag="khi")
        q_lo = const.tile([128, NCHUNK], bf16, tag="qlo")
        q_hi = const.tile([128, NCHUNK], bf16, tag="qhi")
        # replicas shifted by +32 partitions so consecutive blocks of one head
        # use different PE row groups (4-way concurrent scores)
        k_lo2 = const.tile([128, N], bf16, tag="klo2")
        k_hi2 = const.tile([128, N], bf16, tag="khi2")
        q_lo2 = const.tile([128, NCHUNK], bf16, tag="qlo2")
        q_hi2 = const.tile([128, NCHUNK], bf16, tag="qhi2")
        # v'^T: [m-in-block, block, head-half, 65*hh + (64 v cols + ones col)]
        vT_sb = const.tile([128, NB, 2, 260], bf16, tag="vt")
        of_sb = const.tile([64, 8, NCHUNK], bf16, tag="of")
        y_sb = const.tile([128, 2, NCHUNK], f32, tag="y")

        for kc in range(2):
            ks, off = KS[kc], 128 * kc
            nc.sync.dma_start(out=wq_sb[0:ks, kc, :], in_=wq_d[off:off + ks, :])
            nc.sync.dma_start(out=wk_sb[0:ks, kc, :], in_=wk_d[off:off + ks, :])
            nc.sync.dma_start(out=wv_sb[0:ks, kc, :], in_=wv_d[off:off + ks, :])
            nc.sync.dma_start(out=xq_sb[0:ks, kc, :], in_=xq_d[off:off + ks, :])
        nc.sync.dma_start(out=wp_sb[:], in_=wp_d[:])
        nc.sync.dma_start(out=qb_sb[:], in_=qb_d)
        nc.sync.dma_start(out=kb_sb[:], in_=kb_d)
        nc.sync.dma_start(out=pb_sb[:], in_=pb_d)
        nc.gpsimd.dma_start(out=vb_sb[:, :, :],
                            in_=vb_d.partition_broadcast(128))

        with tc.tile_pool(name="stageA", bufs=2) as sA, \
             tc.tile_pool(name="psA", bufs=2, space="PSUM") as psA, \
             tc.tile_pool(name="psAV", bufs=3, space="PSUM") as psAV, \
             tc.tile_pool(name="tmpA", bufs=1) as tA:
            k_sb = tA.tile([128, N], bf16, tag="ksb")
            q_sb = tA.tile([128, NCHUNK], bf16, tag="qsb")
            # q projection (bias folded via ones row of xq)
            for c2 in range(2):
                q_ps = psA.tile([128, 512], f32, tag="qkps")
                for kc in range(2):
                    nc.tensor.matmul(
                        q_ps[0:128, 0:NSUB],
                        wq_sb[0:KS[kc], kc, :],
                        xq_sb[0:KS[kc], kc, c2 * NSUB:(c2 + 1) * NSUB],
                        start=(kc == 0), stop=(kc == 1))
                nc.vector.tensor_scalar_add(
                    q_sb[:, c2 * NSUB:(c2 + 1) * NSUB], q_ps[0:128, 0:NSUB],
                    qb_sb[:, 0:1])
            # k projection + v'^T, in 512-column passes over xa
            for p in range(7):
                c0 = 512 * p
                cw = min(512, N - c0)
                xa_t = sA.tile([128, 2, 512], bf16, tag="xat")
                for kc in range(2):
                    nc.sync.dma_start(
                        out=xa_t[0:KS[kc], kc, 0:cw],
                        in_=xa_d[128 * kc:128 * kc + KS[kc], c0:c0 + cw])
                k_ps = psA.tile([128, 512], f32, tag="qkps")
                for kc in range(2):
                    nc.tensor.matmul(
                        k_ps[0:128, 0:cw],
                        wk_sb[0:KS[kc], kc, :],
                        xa_t[0:KS[kc], kc, 0:cw],
                        start=(kc == 0), stop=(kc == 1))
                if p % 2 == 0:
                    nc.vector.tensor_scalar_add(
                        k_sb[:, c0:c0 + cw], k_ps[0:128, 0:cw], kb_sb[:, 0:1])
                else:
                    nc.scalar.add(k_sb[:, c0:c0 + cw], k_ps[0:128, 0:cw],
                                  kb_sb[:, 0:1])
                for mbi in range(4):
                    mb = 4 * p + mbi
                    if mb >= NB:
                        break
                    pb = MB_SIZES[mb]
                    vt_ps = psAV.tile([128, 2, 512], f32, tag="vtps")
                    for half in range(2):
                        for kc in range(2):
                            nc.tensor.matmul(
                                vt_ps[0:pb, half, 0:260],
                                xa_t[0:KS[kc], kc, mbi * 128:mbi * 128 + pb],
                                wv_sb[0:KS[kc], kc, half * 260:(half + 1) * 260],
                                start=(kc == 0), stop=(kc == 1))
                    nc.vector.tensor_add(
                        out=vT_sb[0:pb, mb, :, :], in0=vt_ps[0:pb, :, 0:260],
                        in1=vb_sb[0:pb, :, :])
            # regroup heads onto 32-aligned bases
            for h in range(8):
                kt = k_lo if h < 4 else k_hi
                qt = q_lo if h < 4 else q_hi
                kt2 = k_lo2 if h < 4 else k_hi2
                qt2 = q_lo2 if h < 4 else q_hi2
                bp_ = 32 * (h % 4)
                bp2 = (bp_ + 32) % 128
                nc.sync.dma_start(out=kt[bp_:bp_ + 16, :], in_=k_sb[16 * h:16 * h + 16, :])
                nc.sync.dma_start(out=qt[bp_:bp_ + 16, :], in_=q_sb[16 * h:16 * h + 16, :])
                nc.sync.dma_start(out=kt2[bp2:bp2 + 16, :], in_=k_sb[16 * h:16 * h + 16, :])
                nc.sync.dma_start(out=qt2[bp2:bp2 + 16, :], in_=q_sb[16 * h:16 * h + 16, :])

        # main attention loop, software-pipelined:
        # iteration i = (head-pair, n-chunk). During iteration i's scores+exp
        # phase, the PE executes iteration i-1's o'-accumulation matmuls as
        # filler, so it never idles waiting on ACT (keeps HAM warm).
        # PSUM: scores 2 slots x 3 banks + o' 2 slots x 1 bank = 8 banks.
        PAIRS = [(0, 2), (1, 3), (4, 6), (5, 7)]
        ITERS = [(pair, c2) for pair in PAIRS for c2 in range(2)]

        def emit_scores_group(pair, c2, blocks, s_ps2, kts, qts, kts2, qts2,
                              bps, bps2):
            nc0 = c2 * NSUB
            for i, mb in enumerate(blocks):
                pbi = MB_SIZES[mb]
                for e in range(2):
                    if mb % 2 == 0:
                        kte, qte, be = kts[e], qts[e], bps[e]
                    else:
                        kte, qte, be = kts2[e], qts2[e], bps2[e]
                    nc.tensor.matmul(
                        s_ps2[e][0:pbi, i, 0:NSUB],
                        kte[be:be + 16, mb * 128:mb * 128 + pbi],
                        qte[be:be + 16, nc0:nc0 + NSUB],
                        start=True, stop=True,
                        tile_position=(be, 0))

        def emit_filler(job):
            # one o'-accumulation matmul of the previous iteration
            (pair, c2, e, p_tile, i, mb, o_ps2) = job
            h = pair[e]
            pbi = MB_SIZES[mb]
            nc.tensor.matmul(
                o_ps2[e][0:65, 0:NSUB],
                vT_sb[0:pbi, mb, h // 4, 65 * (h % 4):65 * (h % 4) + 65],
                p_tile[0:pbi, i, :],
                start=(mb == 0), stop=(mb == NB - 1))

        def emit_epilogue(pair, c2, o_ps2):
            nc0 = c2 * NSUB
            for e in range(2):
                h = pair[e]
                o_ps = o_ps2[e]
                idx = h * 2 + c2
                rsh = pEp.tile([128, NSUB], f32, tag="rsh")
                nc.vector.tensor_copy(rsh[64:65, :], o_ps[64:65, 0:NSUB])
                nc.sync.dma_start(out=rsd_d[idx:idx + 1, :], in_=rsh[64:65, :])
                rb = pEp.tile([64, NSUB], f32, tag="rb")
                nc.sync.dma_start(
                    out=rb[0:64, :],
                    in_=rsd_d[idx:idx + 1, :].partition_broadcast(64))
                rbr = pEp.tile([64, NSUB], f32, tag="rbr")
                scr = pEp.tile([64, NSUB], f32, tag="scr")
                nc.vector.reciprocal_approx_accurate(
                    out=rbr[:], in_=rb[0:64, :], scratch=scr[:])
                nc.vector.tensor_mul(
                    out=of_sb[0:64, h, nc0:nc0 + NSUB],
                    in0=o_ps[0:64, 0:NSUB], in1=rbr[:])

        with tc.tile_pool(name="pP", bufs=22) as pP, \
             tc.tile_pool(name="pEp", bufs=3) as pEp, \
             tc.tile_pool(name="psS", bufs=2, space="PSUM") as psS, \
             tc.tile_pool(name="psO", bufs=2, space="PSUM") as psO:
            prev = None  # (pair, c2, p_tiles) of the previous iteration
            for it in range(len(ITERS) + 1):
                cur = ITERS[it] if it < len(ITERS) else None
                fillers = []
                if prev is not None:
                    ppair, pc2, p_tiles = prev
                    o_ps2 = [psO.tile([128, 512], f32, tag="ops",
                                      name=f"ops{e}") for e in range(2)]
                    for mb in range(NB):
                        g, i = mb // 3, mb % 3
                        for e in range(2):
                            fillers.append((ppair, pc2, e, p_tiles[g][e],
                                            i, mb, o_ps2))
                if cur is None:
                    for job in fillers:
                        emit_filler(job)
                    emit_epilogue(ppair, pc2, o_ps2)
                    break
                pair, c2 = cur
                kts = [k_lo if h < 4 else k_hi for h in pair]
                qts = [q_lo if h < 4 else q_hi for h in pair]
                kts2 = [k_lo2 if h < 4 else k_hi2 for h in pair]
                qts2 = [q_lo2 if h < 4 else q_hi2 for h in pair]
                bps = [32 * (h % 4) for h in pair]
                bps2 = [(32 * (h % 4) + 32) % 128 for h in pair]
                p_tiles = []
                nfill = len(fillers)
                for g, blocks in enumerate(GROUPS):
                    gsz = len(blocks)
                    pb = MB_SIZES[blocks[-1]]
                    s_ps2 = [psS.tile([128, 3, 512], f32, tag="sps",
                                      name=f"sps{e}") for e in range(2)]
                    emit_scores_group(pair, c2, blocks, s_ps2, kts, qts,
                                      kts2, qts2, bps, bps2)
                    p_sb2 = [pP.tile([128, 3, NSUB], bf16, tag="psb",
                                     name=f"psb{e}") for e in range(2)]
                    for e in range(2):
                        nc.scalar.activation(
                            out=p_sb2[e][0:pb, 0:gsz, :],
                            in_=s_ps2[e][0:pb, 0:gsz, 0:NSUB], func=Exp)
                    p_tiles.append(p_sb2)
                    # interleave previous iteration's o' matmuls as PE filler
                    lo = nfill * g // len(GROUPS)
                    hi = nfill * (g + 1) // len(GROUPS)
                    for job in fillers[lo:hi]:
                        emit_filler(job)
                if prev is not None:
                    emit_epilogue(ppair, pc2, o_ps2)
                prev = (pair, c2, p_tiles)

        # output projection: out = wp_eff @ [o; 1]
        with tc.tile_pool(name="psY", bufs=2, space="PSUM") as psY:
            for mo in range(2):
                for c2 in range(2):
                    nc0 = c2 * NSUB
                    y_ps = psY.tile([128, 512], f32, tag="yps")
                    for kc in range(8):
                        nc.tensor.matmul(
                            y_ps[0:128, 0:NSUB],
                            wp_sb[0:64, kc, mo * 128:(mo + 1) * 128],
                            of_sb[0:64, kc, nc0:nc0 + NSUB],
                            start=(kc == 0), stop=(kc == 7))
                    nc.vector.tensor_scalar_add(
                        y_sb[:, mo, nc0:nc0 + NSUB], y_ps[0:128, 0:NSUB],
                        pb_sb[:, mo:mo + 1])
            for mo in range(2):
                nc.sync.dma_start(
                    out=out_d[mo * 128:(mo + 1) * 128, :], in_=y_sb[:, mo, :])

    nc.compile()
    return nc


def get_graph():
    global _GRAPH
    if _GRAPH is None:
        _GRAPH = _build_graph()
    return _GRAPH


def make_in_maps(x, wq, sq, bq, wk, sk, bk, wv, sv, bv, wp, sp, bp):
    import ml_dtypes
    bf = ml_dtypes.bfloat16
    f = np.float32
    x2 = np.asarray(x, f).reshape(B, C, N)
    ones_row = np.ones((1, N), f)
    wq = np.asarray(wq, f); sq = np.asarray(sq, f); bq = np.asarray(bq, f)
    wk = np.asarray(wk, f); sk = np.asarray(sk, f); bk = np.asarray(bk, f)
    wv = np.asarray(wv, f); sv = np.asarray(sv, f); bv = np.asarray(bv, f)
    wp = np.asarray(wp, f); sp = np.asarray(sp, f); bp = np.asarray(bp, f)

    wq_eff = (wq * sq[:, None]).T.astype(f)           # (256, 128)
    wk_eff = (wk * sk[:, None]).T.astype(f)
    wv_base = wv * sv[:, None]  # (512, 256)
    wv_arr = np.zeros((256, 520), f)
    vb_arr = np.zeros((1, 520), f)
    for h in range(NUM_HEADS):
        col = 260 * (h // 4) + 65 * (h % 4)
        wv_arr[:, col:col + 64] = wv_base[64 * h:64 * h + 64, :].T
        vb_arr[0, col:col + 64] = bv[64 * h:64 * h + 64]
        vb_arr[0, col + 64] = 1.0
    wp_eff = (wp * sp[:, None]).T.astype(f)  # (512, 256), row c = 64h+d
    wp_arr = wp_eff.reshape(8, 64, 256).transpose(1, 0, 2).copy()
    pb_arr = bp.reshape(2, 128).T.copy()  # (128, 2): pb_arr[d, mo] = bp[128*mo+d]
    in_maps = []
    for core in range(8):
        b, j = core // 4, core % 4
        xa_full = np.ascontiguousarray(x2[b])
        xq_c = np.ascontiguousarray(xa_full[:, j * NCHUNK:(j + 1) * NCHUNK])
        in_maps.append(dict(
            xa=xa_full.astype(bf), xq=xq_c.astype(bf),
            wq=wq_eff.astype(bf), wk=wk_eff.astype(bf),
            wv=wv_arr.astype(bf), wp=wp_arr.astype(bf),
            qb=bq.reshape(128, 1).astype(f), kb=bk.reshape(128, 1).astype(f),
            vb=vb_arr.astype(bf), pb=pb_arr.astype(f)))
    return in_maps


def assemble_output(results):
    y = np.zeros((B, C, N), np.float32)
    for core in range(8):
        b, j = core // 4, core % 4
        y[b, :, j * NCHUNK:(j + 1) * NCHUNK] = results[core]["out"]
    return y.reshape(B, C, HH, WW)


def kernel(**inputs):
    from concourse.bass_utils import run_bass_kernel_spmd
    nc = get_graph()
    in_maps = make_in_maps(**inputs)
    res = run_bass_kernel_spmd(nc, in_maps, core_ids=list(range(8)))
    return assemble_output(res.results)


if __name__ == "__main__":
    rng = np.random.default_rng(0)
    ins = dict(
        x=rng.standard_normal((2, 256, 56, 56), np.float32),
        wq=rng.standard_normal((128, 256), np.float32) * 0.05,
        sq=rng.random(128, np.float32),
        bq=rng.standard_normal(128, np.float32) * 0.05,
        wk=rng.standard_normal((128, 256), np.float32) * 0.05,
        sk=rng.random(128, np.float32),
        bk=rng.standard_normal(128, np.float32) * 0.05,
        wv=rng.standard_normal((512, 256), np.float32) * 0.05,
        sv=rng.random(512, np.float32),
        bv=rng.standard_normal(512, np.float32) * 0.05,
        wp=rng.standard_normal((256, 512), np.float32) * 0.05,
        sp=rng.random(256, np.float32),
        bp=rng.standard_normal(256, np.float32) * 0.05,
    )
    out = kernel(**ins)
    print("out", out.shape, out.dtype, float(np.abs(out).mean()))
